# revision 21
# baseline (speedup 1.0000x reference)
"""KimiSparseMoeBlock kernel for 8 Trainium2 NeuronCores.

Sharding (expert-parallel, per spec hint):
  - 32 experts sharded 4-per-core (w1/w2/w3 leading dim), with the
    expert->core assignment chosen at runtime from the actual routed
    load: experts are sorted by token count and dealt into 4 "bands"
    (8 experts each, one per core).  Band j's static column capacity is
    the max count within the band, so the compiled SPMD program only
    computes (close to) the real routed tokens instead of the full
    2x-mean capacity buffer the reference uses.
  - Shared SwiGLU tensor-parallel along FS (2048/8 = 256 per core).
  - Token dispatch/combine done ON DEVICE via dma_gather(transpose) /
    dma_scatter_add with per-core index lists; gate/top-k routing
    metadata is computed host-side during input sharding (~0.1% FLOPs).
  - Each core emits two partial outputs (scatter-accumulated MoE rows
    and the FS-shard of the shared expert); unshard = sum of partials.

Device pipeline per core (per expert slot j with column cap S_j):
  gather ceil(S_j/128)*128 token rows (bf16, transposed to [H, W]) ->
  SwiGLU GEMMs on exactly S_j columns (bf16 PE, fp32 PSUM) ->
  gating-scaled rows -> dma_scatter_add into y_moe; plus FS-sharded
  shared SwiGLU from host-pretransposed xT.
"""
import os
import sys

sys.path.insert(0, "/opt/trn_rl_repo")

import numpy as np
import ml_dtypes

import concourse.bass as bass
import concourse.bacc as bacc
import concourse.tile as tile
import concourse.mybir as mybir
from concourse.bass_utils import run_bass_kernel_spmd

F32 = mybir.dt.float32
BF16 = mybir.dt.bfloat16
I16 = mybir.dt.int16
BF = ml_dtypes.bfloat16

E, K, G, TG = 32, 4, 4, 2
H, F, FS = 2048, 1024, 2048
SCALE = 2.0
B, S = 2, 1024
N = B * S
CAP = 2 * (N * K // E)  # 512 (reference capacity; tokens beyond it drop)
NCORES = 8
EPC = E // NCORES       # expert slots per core = 4
FSS = FS // NCORES      # shared intermediate per core = 256
HC = H // 128           # 16 contraction chunks
FC = F // 128           # 8 F chunks


# ---------------------------------------------------------------- routing
def _gate_host(x, gate_w, gate_bias):
    """Replicate reference _gate in numpy (f32, jax-compatible ops)."""
    x = x.astype(np.float32)
    logits = x @ gate_w.T.astype(np.float32)
    scores = 1.0 / (1.0 + np.exp(-logits))
    sc = scores + gate_bias[None, :]
    n = x.shape[0]
    grp = sc.reshape(n, G, E // G)
    top2 = np.sort(grp, axis=-1)[:, :, -2:]
    group_scores = top2.sum(-1)
    gidx = np.argsort(-group_scores, axis=-1, kind="stable")[:, :TG]
    gmask = np.zeros((n, G), np.float32)
    np.put_along_axis(gmask, gidx, 1.0, axis=1)
    smask = np.repeat(gmask, E // G, axis=1)
    tmp = np.where(smask > 0, sc, 0.0)
    topk_idx = np.argsort(-tmp, axis=-1, kind="stable")[:, :K].astype(np.int32)
    topk_w = np.take_along_axis(scores, topk_idx, axis=1)
    topk_w = topk_w / (topk_w.sum(-1, keepdims=True) + 1e-20)
    return topk_idx, (topk_w * SCALE).astype(np.float32)


def _gate_host_jax(x, gate_w, gate_bias):
    """Bit-exact replication of the reference gate on jax-cpu."""
    try:
        import jax

        import jax.numpy as jnp

        def gate(x, gate_w, gate_bias):
            logits = x @ gate_w.T
            scores = jax.nn.sigmoid(logits)
            sc = scores + gate_bias
            n = x.shape[0]
            grp = sc.reshape(n, G, E // G)
            group_scores = jax.lax.top_k(grp, 2)[0].sum(-1)
            _, gidx = jax.lax.top_k(group_scores, TG)
            gmask = (
                jnp.zeros((n, G), sc.dtype)
                .at[jnp.arange(n)[:, None], gidx]
                .set(1.0)
            )
            smask = jnp.repeat(gmask, E // G, axis=1)
            tmp = jnp.where(smask > 0, sc, 0.0)
            _, topk_idx = jax.lax.top_k(tmp, K)
            topk_w = jnp.take_along_axis(scores, topk_idx, axis=1)
            topk_w = topk_w / (topk_w.sum(-1, keepdims=True) + 1e-20)
            return topk_idx, topk_w * SCALE

        with jax.default_device(jax.devices("cpu")[0]):
            ti, tw = jax.jit(gate, backend="cpu")(x, gate_w, gate_bias)
        return np.asarray(ti, np.int32), np.asarray(tw, np.float32)
    except Exception:
        return _gate_host(x, gate_w, gate_bias)


def _wrap_idx(idx):
    """[n] -> [128, n//16]: slot s at partition s%16 (replicated x8), col s//16."""
    n = idx.shape[0]
    w = idx.reshape(n // 16, 16).T
    return np.tile(w, (8, 1)).copy()


# ---------------------------------------------------------------- bass build
_CACHE = {}


def _build(sched):
    """sched = (scaps, nblks): static per-slot column caps / 128-blocks."""
    if sched in _CACHE:
        return _CACHE[sched]
    scaps, nblks = sched
    NBT = sum(nblks)           # total 128-row token blocks per core
    SLOTS_TOT = 128 * NBT

    nc = bacc.Bacc("TRN2", target_bir_lowering=False, debug=False,
                   num_devices=NCORES)
    t_x = nc.dram_tensor("x_nat", [N + 1, H], BF16, kind="ExternalInput")
    t_xT = nc.dram_tensor("xT", [128, H // 128, N], BF16, kind="ExternalInput")
    t_idx = nc.dram_tensor("idx_w", [128, SLOTS_TOT // 16], I16,
                           kind="ExternalInput")
    t_gat = nc.dram_tensor("gat_col", [128, NBT], F32, kind="ExternalInput")
    t_w1 = nc.dram_tensor("w1", [EPC, F // 128, 128, H], BF16, kind="ExternalInput")
    t_w3 = nc.dram_tensor("w3", [EPC, F // 128, 128, H], BF16, kind="ExternalInput")
    t_w2 = nc.dram_tensor("w2", [EPC, 128, F // 128, H], BF16, kind="ExternalInput")
    t_sg = nc.dram_tensor("sg", [128, H // 128, FSS], BF16, kind="ExternalInput")
    t_su = nc.dram_tensor("su", [128, H // 128, FSS], BF16, kind="ExternalInput")
    t_sd = nc.dram_tensor("sd", [128, FSS // 128, H], BF16, kind="ExternalInput")
    t_ymoe = nc.dram_tensor("y_moe", [N + 1, H], BF16, kind="ExternalOutput")
    t_ysh = nc.dram_tensor("y_sh", [N, H], BF16, kind="ExternalOutput")

    coffs = np.cumsum([0] + [8 * nb for nb in nblks]).tolist()   # idx col offs
    boffs = np.cumsum([0] + list(nblks)).tolist()                # block offs

    with tile.TileContext(nc) as tc:
        with (
            tc.tile_pool(name="idxp", bufs=1) as idxp,
            tc.tile_pool(name="gath", bufs=2) as gath,
            tc.tile_pool(name="wp", bufs=3) as wp,
            tc.tile_pool(name="w2p", bufs=2) as w2p,
            tc.tile_pool(name="hp", bufs=2) as hp,
            tc.tile_pool(name="obp", bufs=2) as obp,
            tc.tile_pool(name="shp", bufs=1) as shp,
            tc.tile_pool(name="ps", bufs=2, space="PSUM") as ps,
        ):
            # idx on the pool queue (its consumer): the first gather starts
            # as soon as it lands instead of queueing behind w2 chunks.
            idx_t = idxp.tile([128, SLOTS_TOT // 16], I16)
            nc.gpsimd.dma_start(idx_t[:], t_idx[:])
            gat_t = idxp.tile([128, NBT], F32)   # load deferred (needed ~30us in)

            bufs = [None] * EPC   # gathered token tiles (list per slot)
            w2s = [None] * EPC    # w2 weight tiles
            xtts = [None] * 4     # shared-path xT tiles

            def emit_gather(j, split=1):
                # split>1: issue the gather as `split` pieces so the first
                # GEMM can start before the whole slot is resident.
                W = 128 * nblks[j]
                tiles = []
                step = W // split
                assert step % 128 == 0
                for s in range(split):
                    t = gath.tile([128, HC, step], BF16, tag="bufT",
                                  name=f"bufT{j}_{s}")
                    c0 = coffs[j] + s * step // 16
                    nc.gpsimd.dma_gather(
                        t[:], t_x[:], idx_t[:, c0: c0 + step // 16],
                        step, step, H, transpose=True,
                    )
                    tiles.append(t)
                bufs[j] = tiles

            def emit_w2(j, split=1):
                # chunked per-f so urgent ops never queue behind a 4MB DMA;
                # split=2 halves the chunks again (startup-critical window)
                w2s[j] = w2p.tile([128, FC, H], BF16, tag="w2e",
                                  name=f"w2e{j}")
                hstep = H // split
                for f in range(FC):
                    for s in range(split):
                        nc.gpsimd.dma_start(
                            w2s[j][:, f, hstep * s: hstep * (s + 1)],
                            t_w2[j, :, f, hstep * s: hstep * (s + 1)],
                        )

            def emit_xtt(tb):
                xtts[tb] = gath.tile([128, HC, 512], BF16, tag="xTt",
                                     name=f"xTt{tb}")
                for s in range(4):
                    nc.sync.dma_start(
                        xtts[tb][:, :, 128 * s: 128 * (s + 1)],
                        t_xT[:, :, 512 * tb + 128 * s: 512 * tb + 128 * (s + 1)],
                    )

            # Prefetch: slot 0/1 gathers + slot 0/1 w2 up-front (pool queue);
            # slot j+2's gather/w2 are emitted at slot j+1's head so they sit
            # behind slot j's scatters but well ahead of their consumers.
            with tc.high_priority():
                emit_gather(0, split=2 if nblks[0] >= 2 else 1)
                emit_gather(1)
            emit_w2(0, split=2)
            emit_w2(1)

            # ---------------- expert path ----------------
            for j in range(EPC):
                Sj = scaps[j]
                W = 128 * nblks[j]
                if j >= 1 and j + 1 < EPC:
                    emit_gather(j + 1)
                    emit_w2(j + 1)
                # clip gather pieces to the Sj real columns
                pieces = []
                off = 0
                for t in bufs[j]:
                    w = t.shape[2]
                    lo = off
                    hi = min(off + w, Sj)
                    if hi > lo:
                        pieces.append((t, lo, hi - lo))
                    off += w
                # GEMM1 + GEMM3 + SwiGLU -> hT [128, FC, W] bf16
                hT = hp.tile([128, FC, W], BF16, tag="hT", name=f"hT{j}")
                if Sj < W:
                    nc.vector.memset(hT[:, :, Sj:], 0.0)
                for f in range(FC):
                    w1f = wp.tile([128, HC, 128], BF16, tag="w1f")
                    w3f = wp.tile([128, HC, 128], BF16, tag="w3f")
                    # first w1 of the run goes via SP — the Act queue opens
                    # with a 1.3us act-table load
                    w1eng = nc.sync if (j == 0 and f == 0) else nc.scalar
                    w1eng.dma_start(
                        w1f[:], t_w1[j, f].rearrange("p (hc fo) -> p hc fo", hc=HC)
                    )
                    nc.sync.dma_start(
                        w3f[:], t_w3[j, f].rearrange("p (hc fo) -> p hc fo", hc=HC)
                    )
                    p1 = ps.tile([128, Sj], F32, tag="p1")
                    p3 = ps.tile([128, Sj], F32, tag="p3")
                    for (t, lo, n) in pieces:
                        for h in range(HC):
                            nc.tensor.matmul(p1[:, lo: lo + n], w1f[:, h, :],
                                             t[:, h, :n],
                                             start=(h == 0), stop=(h == HC - 1))
                    for (t, lo, n) in pieces:
                        for h in range(HC):
                            nc.tensor.matmul(p3[:, lo: lo + n], w3f[:, h, :],
                                             t[:, h, :n],
                                             start=(h == 0), stop=(h == HC - 1))
                    sig = hp.tile([128, Sj], F32, tag="sig")
                    nc.scalar.activation(
                        sig[:], p1[:], mybir.ActivationFunctionType.Sigmoid
                    )
                    nc.vector.tensor_tensor(
                        sig[:], sig[:], p1[:], op=mybir.AluOpType.mult
                    )
                    nc.vector.tensor_tensor(
                        hT[:, f, :Sj], sig[:], p3[:], op=mybir.AluOpType.mult
                    )
                if j == 0:
                    nc.sync.dma_start(gat_t[:], t_gat[:])
                # prefetch shared-path xT during the last slot's GEMM2
                if j == EPC - 1:
                    emit_xtt(0)
                    emit_xtt(1)
                # GEMM2 + gating scale -> ob [128, H] bf16, then scatter
                w2e = w2s[j]
                for rb in range(nblks[j]):
                    ob = obp.tile([128, 1, H], BF16, tag="ob")
                    for hc in range(H // 512):
                        p2 = ps.tile([128, 512], F32, tag="p2", bufs=3)
                        for f in range(FC):
                            nc.tensor.matmul(
                                p2[:],
                                hT[:, f, 128 * rb: 128 * (rb + 1)],
                                w2e[:, f, 512 * hc: 512 * (hc + 1)],
                                start=(f == 0), stop=(f == FC - 1),
                            )
                        nc.vector.tensor_scalar_mul(
                            ob[:, 0, 512 * hc: 512 * (hc + 1)], p2[:],
                            gat_t[:, boffs[j] + rb: boffs[j] + rb + 1],
                        )
                    nc.gpsimd.dma_scatter_add(
                        t_ymoe[:], ob[:],
                        idx_t[:, coffs[j] + 8 * rb: coffs[j] + 8 * (rb + 1)],
                        128, 128, H,
                    )

            # ---------------- shared expert (FS shard) ----------------
            sgt = shp.tile([128, HC, FSS], BF16, tag="sgt")
            sut = shp.tile([128, HC, FSS], BF16, tag="sut")
            sdt = shp.tile([128, FSS // 128, H], BF16, tag="sdt")
            for h2 in range(0, HC, 8):       # <=1.6us chunks
                nc.scalar.dma_start(sgt[:, h2: h2 + 8, :], t_sg[:, h2: h2 + 8, :])
                nc.scalar.dma_start(sut[:, h2: h2 + 8, :], t_su[:, h2: h2 + 8, :])
            for fs2 in range(FSS // 128):
                nc.scalar.dma_start(sdt[:, fs2: fs2 + 1, :], t_sd[:, fs2: fs2 + 1, :])
            for tb in range(N // 512):
                if tb + 2 < 4:
                    emit_xtt(tb + 2)
                xTt = xtts[tb]
                ttT = hp.tile([128, FSS // 128, 512], BF16, tag="ttT")
                for fs in range(FSS // 128):
                    pg = ps.tile([128, 512], F32, tag="p1")
                    pu = ps.tile([128, 512], F32, tag="p3")
                    for h in range(HC):
                        nc.tensor.matmul(
                            pg[:], sgt[:, h, 128 * fs: 128 * (fs + 1)],
                            xTt[:, h, :], start=(h == 0), stop=(h == HC - 1),
                        )
                    for h in range(HC):
                        nc.tensor.matmul(
                            pu[:], sut[:, h, 128 * fs: 128 * (fs + 1)],
                            xTt[:, h, :], start=(h == 0), stop=(h == HC - 1),
                        )
                    sig = hp.tile([128, 512], F32, tag="sigsh")
                    nc.scalar.activation(
                        sig[:], pg[:], mybir.ActivationFunctionType.Sigmoid
                    )
                    nc.vector.tensor_tensor(
                        sig[:], sig[:], pg[:], op=mybir.AluOpType.mult
                    )
                    nc.vector.tensor_tensor(
                        ttT[:, fs, :], sig[:], pu[:], op=mybir.AluOpType.mult
                    )
                for ts in range(4):
                    osh = obp.tile([128, 1, H], BF16, tag="ob")
                    for hc in range(H // 512):
                        p2 = ps.tile([128, 512], F32, tag="p2", bufs=3)
                        for fs in range(FSS // 128):
                            nc.tensor.matmul(
                                p2[:],
                                ttT[:, fs, 128 * ts: 128 * (ts + 1)],
                                sdt[:, fs, 512 * hc: 512 * (hc + 1)],
                                start=(fs == 0), stop=(fs == FSS // 128 - 1),
                            )
                        # split the PSUM drain across DVE and Act so the
                        # copy rate beats the GEMM produce rate
                        nc.vector.tensor_copy(
                            osh[:, 0, 512 * hc: 512 * hc + 256], p2[:, :256]
                        )
                        nc.scalar.activation(
                            osh[:, 0, 512 * hc + 256: 512 * (hc + 1)],
                            p2[:, 256:],
                            mybir.ActivationFunctionType.Copy,
                        )
                        # store each H-half as soon as its copies land,
                        # alternating the two hwdge queues
                        if hc % 2 == 1:
                            eng = nc.sync if hc == 1 else nc.scalar
                            eng.dma_start(
                                t_ysh[512 * tb + 128 * ts:
                                      512 * tb + 128 * (ts + 1),
                                      1024 * (hc // 2): 1024 * (hc // 2 + 1)],
                                osh[:, 0, 1024 * (hc // 2): 1024 * (hc // 2 + 1)],
                            )
    nc.compile()
    _CACHE[sched] = nc
    return nc


# ---------------------------------------------------------------- host glue
def _prep_inputs(hidden_states, gate_w, gate_bias, w1, w2, w3,
                 shared_gate, shared_up, shared_down):
    x = np.asarray(hidden_states, np.float32).reshape(N, H)

    def tile_lhsT(w):
        # [E, H, F] -> [E, FC, 128(h-part), HC*128]: tile (e,f)[p, hc*128+fo]
        # = w[e, 128*hc + p, 128*f + fo]
        we = w.reshape(-1, H // 128, 128, F // 128, 128)
        return np.ascontiguousarray(we.transpose(0, 3, 2, 1, 4)).reshape(
            -1, F // 128, 128, H)

    def tile_rhs(w, kc):
        # [E?, KC*128, M] -> [..., 128(part), KC, M]: (p, kc, m) = w[128*kc+p, m]
        wr = w.reshape(-1, kc, 128, w.shape[-1])
        return np.ascontiguousarray(wr.transpose(0, 2, 1, 3)).reshape(
            -1, 128, kc * w.shape[-1])

    topk_idx, topk_w = _gate_host_jax(x, np.asarray(gate_w, np.float32),
                                      np.asarray(gate_bias, np.float32))

    # capacity dispatch identical to reference: pos = per-expert running slot
    flat_e = topk_idx.reshape(-1)
    pos = np.zeros(N * K, np.int64)
    cnt = np.zeros(E, np.int64)
    for i, e in enumerate(flat_e):
        pos[i] = cnt[e]
        cnt[e] += 1
    keep = pos < CAP
    kept_counts = np.minimum(cnt, CAP)

    # Band schedule: sort experts by routed load (ascending, so the first
    # slot's gather is the smallest -> shortest startup), deal 8 per band
    # (one per core).  Static per-slot column cap = band max (padded to 4).
    order = np.argsort(kept_counts, kind="stable")
    scaps, nblks = [], []
    for j in range(EPC):
        cap = int(kept_counts[order[8 * j: 8 * (j + 1)]].max())
        cap = max(4, (cap + 3) // 4 * 4)
        scaps.append(cap)
        nblks.append((cap + 127) // 128)
    sched = (tuple(scaps), tuple(nblks))
    NBT = sum(nblks)
    SLOTS_TOT = 128 * NBT

    x_nat = np.zeros((N + 1, H), BF)
    x_nat[:N] = x.astype(BF)
    xb = x.astype(BF)
    xT = tile_rhs(xb.T, H // 128)[0].reshape(128, H // 128, N)
    w1b = tile_lhsT(np.asarray(w1, np.float32).astype(BF))
    w3b = tile_lhsT(np.asarray(w3, np.float32).astype(BF))
    w2b = tile_rhs(np.asarray(w2, np.float32).astype(BF), F // 128).reshape(
        E, 128, F // 128, H)
    sgb = np.asarray(shared_gate, np.float32).astype(BF)
    sub = np.asarray(shared_up, np.float32).astype(BF)
    sdb = np.asarray(shared_down, np.float32).astype(BF)
    tw_flat = topk_w.reshape(-1)

    in_maps = []
    for c in range(NCORES):
        idx = np.full(SLOTS_TOT, N, np.int16)  # pads -> dump row N
        gat = np.zeros(SLOTS_TOT, np.float32)  # pads -> weight 0
        experts = []
        off = 0
        for j in range(EPC):
            eg = int(order[8 * j + c])
            experts.append(eg)
            sel = np.nonzero((flat_e == eg) & keep)[0]
            idx[off: off + len(sel)] = sel // K
            gat[off: off + len(sel)] = tw_flat[sel]
            off += 128 * nblks[j]
        lo = c * FSS
        in_maps.append({
            "x_nat": x_nat,
            "xT": xT,
            "idx_w": _wrap_idx(idx),
            "gat_col": gat.reshape(NBT, 128).T.copy(),
            "w1": np.ascontiguousarray(w1b[experts]),
            "w3": np.ascontiguousarray(w3b[experts]),
            "w2": np.ascontiguousarray(w2b[experts]),
            "sg": tile_rhs(sgb[:, lo:lo + FSS], H // 128)[0].reshape(128, H // 128, FSS),
            "su": tile_rhs(sub[:, lo:lo + FSS], H // 128)[0].reshape(128, H // 128, FSS),
            "sd": tile_rhs(sdb[lo:lo + FSS, :], FSS // 128)[0].reshape(128, FSS // 128, H),
        })
    return in_maps, sched


def kernel(hidden_states, gate_w, gate_bias, w1, w2, w3,
           shared_gate, shared_up, shared_down, _trace=False):
    in_maps, sched = _prep_inputs(hidden_states, gate_w, gate_bias, w1, w2, w3,
                                  shared_gate, shared_up, shared_down)
    nc = _build(sched)
    res = run_bass_kernel_spmd(nc, in_maps, list(range(NCORES)), trace=_trace)
    y = np.zeros((N, H), np.float64)
    for r in res.results:
        y += r["y_moe"][:N].astype(np.float64)
        y += r["y_sh"].astype(np.float64)
    out = y.astype(np.float32).reshape(B, S, H)
    if _trace:
        kernel._last = res
    return out


# revision 22
# speedup vs baseline: 1.0061x; 1.0061x over previous
"""KimiSparseMoeBlock kernel for 8 Trainium2 NeuronCores.

Sharding (expert-parallel, per spec hint):
  - 32 experts sharded 4-per-core (w1/w2/w3 leading dim), with the
    expert->core assignment chosen at runtime from the actual routed
    load: experts are sorted by token count and dealt into 4 "bands"
    (8 experts each, one per core).  Band j's static column capacity is
    the max count within the band, so the compiled SPMD program only
    computes (close to) the real routed tokens instead of the full
    2x-mean capacity buffer the reference uses.
  - Shared SwiGLU tensor-parallel along FS (2048/8 = 256 per core).
  - Token dispatch/combine done ON DEVICE via dma_gather(transpose) /
    dma_scatter_add with per-core index lists; gate/top-k routing
    metadata is computed host-side during input sharding (~0.1% FLOPs).
  - Each core emits two partial outputs (scatter-accumulated MoE rows
    and the FS-shard of the shared expert); unshard = sum of partials.

Device pipeline per core (per expert slot j with column cap S_j):
  gather ceil(S_j/128)*128 token rows (bf16, transposed to [H, W]) ->
  SwiGLU GEMMs on exactly S_j columns (bf16 PE, fp32 PSUM) ->
  gating-scaled rows -> dma_scatter_add into y_moe; plus FS-sharded
  shared SwiGLU from host-pretransposed xT.
"""
import os
import sys

sys.path.insert(0, "/opt/trn_rl_repo")

import numpy as np
import ml_dtypes

import concourse.bass as bass
import concourse.bacc as bacc
import concourse.tile as tile
import concourse.mybir as mybir
from concourse.bass_utils import run_bass_kernel_spmd

F32 = mybir.dt.float32
BF16 = mybir.dt.bfloat16
I16 = mybir.dt.int16
BF = ml_dtypes.bfloat16

E, K, G, TG = 32, 4, 4, 2
H, F, FS = 2048, 1024, 2048
SCALE = 2.0
B, S = 2, 1024
N = B * S
CAP = 2 * (N * K // E)  # 512 (reference capacity; tokens beyond it drop)
NCORES = 8
EPC = E // NCORES       # expert slots per core = 4
FSS = FS // NCORES      # shared intermediate per core = 256
HC = H // 128           # 16 contraction chunks
FC = F // 128           # 8 F chunks


# ---------------------------------------------------------------- routing
def _gate_host(x, gate_w, gate_bias):
    """Replicate reference _gate in numpy (f32, jax-compatible ops)."""
    x = x.astype(np.float32)
    logits = x @ gate_w.T.astype(np.float32)
    scores = 1.0 / (1.0 + np.exp(-logits))
    sc = scores + gate_bias[None, :]
    n = x.shape[0]
    grp = sc.reshape(n, G, E // G)
    top2 = np.sort(grp, axis=-1)[:, :, -2:]
    group_scores = top2.sum(-1)
    gidx = np.argsort(-group_scores, axis=-1, kind="stable")[:, :TG]
    gmask = np.zeros((n, G), np.float32)
    np.put_along_axis(gmask, gidx, 1.0, axis=1)
    smask = np.repeat(gmask, E // G, axis=1)
    tmp = np.where(smask > 0, sc, 0.0)
    topk_idx = np.argsort(-tmp, axis=-1, kind="stable")[:, :K].astype(np.int32)
    topk_w = np.take_along_axis(scores, topk_idx, axis=1)
    topk_w = topk_w / (topk_w.sum(-1, keepdims=True) + 1e-20)
    return topk_idx, (topk_w * SCALE).astype(np.float32)


def _gate_host_jax(x, gate_w, gate_bias):
    """Bit-exact replication of the reference gate on jax-cpu."""
    try:
        import jax

        import jax.numpy as jnp

        def gate(x, gate_w, gate_bias):
            logits = x @ gate_w.T
            scores = jax.nn.sigmoid(logits)
            sc = scores + gate_bias
            n = x.shape[0]
            grp = sc.reshape(n, G, E // G)
            group_scores = jax.lax.top_k(grp, 2)[0].sum(-1)
            _, gidx = jax.lax.top_k(group_scores, TG)
            gmask = (
                jnp.zeros((n, G), sc.dtype)
                .at[jnp.arange(n)[:, None], gidx]
                .set(1.0)
            )
            smask = jnp.repeat(gmask, E // G, axis=1)
            tmp = jnp.where(smask > 0, sc, 0.0)
            _, topk_idx = jax.lax.top_k(tmp, K)
            topk_w = jnp.take_along_axis(scores, topk_idx, axis=1)
            topk_w = topk_w / (topk_w.sum(-1, keepdims=True) + 1e-20)
            return topk_idx, topk_w * SCALE

        with jax.default_device(jax.devices("cpu")[0]):
            ti, tw = jax.jit(gate, backend="cpu")(x, gate_w, gate_bias)
        return np.asarray(ti, np.int32), np.asarray(tw, np.float32)
    except Exception:
        return _gate_host(x, gate_w, gate_bias)


def _wrap_idx(idx):
    """[n] -> [128, n//16]: slot s at partition s%16 (replicated x8), col s//16."""
    n = idx.shape[0]
    w = idx.reshape(n // 16, 16).T
    return np.tile(w, (8, 1)).copy()


# ---------------------------------------------------------------- bass build
_CACHE = {}


def _build(sched):
    """sched = (scaps, nblks): static per-slot column caps / 128-blocks."""
    if sched in _CACHE:
        return _CACHE[sched]
    scaps, nblks = sched
    NBT = sum(nblks)           # total 128-row token blocks per core
    SLOTS_TOT = 128 * NBT

    nc = bacc.Bacc("TRN2", target_bir_lowering=False, debug=False,
                   num_devices=NCORES)
    t_x = nc.dram_tensor("x_nat", [N + 1, H], BF16, kind="ExternalInput")
    t_xT = nc.dram_tensor("xT", [128, H // 128, N], BF16, kind="ExternalInput")
    t_idx = nc.dram_tensor("idx_w", [128, SLOTS_TOT // 16], I16,
                           kind="ExternalInput")
    t_gat = nc.dram_tensor("gat_col", [128, NBT], F32, kind="ExternalInput")
    t_w1 = nc.dram_tensor("w1", [EPC, F // 128, 128, H], BF16, kind="ExternalInput")
    t_w3 = nc.dram_tensor("w3", [EPC, F // 128, 128, H], BF16, kind="ExternalInput")
    t_w2 = nc.dram_tensor("w2", [EPC, 128, F // 128, H], BF16, kind="ExternalInput")
    t_sg = nc.dram_tensor("sg", [128, H // 128, FSS], BF16, kind="ExternalInput")
    t_su = nc.dram_tensor("su", [128, H // 128, FSS], BF16, kind="ExternalInput")
    t_sd = nc.dram_tensor("sd", [128, FSS // 128, H], BF16, kind="ExternalInput")
    t_ymoe = nc.dram_tensor("y_moe", [N + 1, H], BF16, kind="ExternalOutput")
    t_ysh = nc.dram_tensor("y_sh", [N, H], BF16, kind="ExternalOutput")

    coffs = np.cumsum([0] + [8 * nb for nb in nblks]).tolist()   # idx col offs
    boffs = np.cumsum([0] + list(nblks)).tolist()                # block offs

    with tile.TileContext(nc) as tc:
        with (
            tc.tile_pool(name="idxp", bufs=1) as idxp,
            tc.tile_pool(name="gath", bufs=2) as gath,
            tc.tile_pool(name="wp", bufs=3) as wp,
            tc.tile_pool(name="w2p", bufs=2) as w2p,
            tc.tile_pool(name="hp", bufs=2) as hp,
            tc.tile_pool(name="obp", bufs=2) as obp,
            tc.tile_pool(name="shp", bufs=1) as shp,
            tc.tile_pool(name="ps", bufs=2, space="PSUM") as ps,
        ):
            # idx on the pool queue (its consumer): the first gather starts
            # as soon as it lands instead of queueing behind w2 chunks.
            idx_t = idxp.tile([128, SLOTS_TOT // 16], I16)
            nc.gpsimd.dma_start(idx_t[:], t_idx[:])
            gat_t = idxp.tile([128, NBT], F32)   # load deferred (needed ~30us in)

            bufs = [None] * EPC   # gathered token tiles (list per slot)
            w2s = [None] * EPC    # w2 weight tiles
            xtts = [None] * 4     # shared-path xT tiles

            def emit_gather(j, split=1):
                # split>1: issue the gather as `split` pieces so the first
                # GEMM can start before the whole slot is resident.
                W = 128 * nblks[j]
                tiles = []
                step = W // split
                assert step % 128 == 0
                for s in range(split):
                    t = gath.tile([128, HC, step], BF16, tag="bufT",
                                  name=f"bufT{j}_{s}")
                    c0 = coffs[j] + s * step // 16
                    nc.gpsimd.dma_gather(
                        t[:], t_x[:], idx_t[:, c0: c0 + step // 16],
                        step, step, H, transpose=True,
                    )
                    tiles.append(t)
                bufs[j] = tiles

            def emit_w2(j, split=1):
                # chunked per-f so urgent ops never queue behind a 4MB DMA;
                # split=2 halves the chunks again (startup-critical window)
                w2s[j] = w2p.tile([128, FC, H], BF16, tag="w2e",
                                  name=f"w2e{j}")
                hstep = H // split
                for f in range(FC):
                    for s in range(split):
                        nc.gpsimd.dma_start(
                            w2s[j][:, f, hstep * s: hstep * (s + 1)],
                            t_w2[j, :, f, hstep * s: hstep * (s + 1)],
                        )

            def emit_xtt(tb):
                xtts[tb] = gath.tile([128, HC, 512], BF16, tag="xTt",
                                     name=f"xTt{tb}")
                for s in range(4):
                    nc.sync.dma_start(
                        xtts[tb][:, :, 128 * s: 128 * (s + 1)],
                        t_xT[:, :, 512 * tb + 128 * s: 512 * tb + 128 * (s + 1)],
                    )

            # Prefetch: slot 0/1 gathers + slot 0/1 w2 up-front (pool queue);
            # slot j+2's gather/w2 are emitted at slot j+1's head so they sit
            # behind slot j's scatters but well ahead of their consumers.
            with tc.high_priority():
                emit_gather(0, split=2 if nblks[0] >= 2 else 1)
                emit_gather(1)
            emit_w2(0, split=2)
            emit_w2(1)

            # ---------------- expert path ----------------
            for j in range(EPC):
                Sj = scaps[j]
                W = 128 * nblks[j]
                if j >= 1 and j + 1 < EPC:
                    emit_gather(j + 1)
                    emit_w2(j + 1)
                # clip gather pieces to the Sj real columns
                pieces = []
                off = 0
                for t in bufs[j]:
                    w = t.shape[2]
                    lo = off
                    hi = min(off + w, Sj)
                    if hi > lo:
                        pieces.append((t, lo, hi - lo))
                    off += w
                # GEMM1 + GEMM3 + SwiGLU -> hT [128, FC, W] bf16
                hT = hp.tile([128, FC, W], BF16, tag="hT", name=f"hT{j}")
                if Sj < W:
                    nc.vector.memset(hT[:, :, Sj:], 0.0)
                for f in range(FC):
                    w1f = wp.tile([128, HC, 128], BF16, tag="w1f")
                    w3f = wp.tile([128, HC, 128], BF16, tag="w3f")
                    # first w1 of the run goes via SP — the Act queue opens
                    # with a 1.3us act-table load
                    w1eng = nc.sync if (j == 0 and f == 0) else nc.scalar
                    w1eng.dma_start(
                        w1f[:], t_w1[j, f].rearrange("p (hc fo) -> p hc fo", hc=HC)
                    )
                    nc.sync.dma_start(
                        w3f[:], t_w3[j, f].rearrange("p (hc fo) -> p hc fo", hc=HC)
                    )
                    p1 = ps.tile([128, Sj], F32, tag="p1")
                    p3 = ps.tile([128, Sj], F32, tag="p3")
                    for (t, lo, n) in pieces:
                        for h in range(HC):
                            nc.tensor.matmul(p1[:, lo: lo + n], w1f[:, h, :],
                                             t[:, h, :n],
                                             start=(h == 0), stop=(h == HC - 1))
                    for (t, lo, n) in pieces:
                        for h in range(HC):
                            nc.tensor.matmul(p3[:, lo: lo + n], w3f[:, h, :],
                                             t[:, h, :n],
                                             start=(h == 0), stop=(h == HC - 1))
                    sig = hp.tile([128, Sj], F32, tag="sig")
                    nc.scalar.activation(
                        sig[:], p1[:], mybir.ActivationFunctionType.Sigmoid
                    )
                    nc.vector.tensor_tensor(
                        sig[:], sig[:], p1[:], op=mybir.AluOpType.mult
                    )
                    nc.vector.tensor_tensor(
                        hT[:, f, :Sj], sig[:], p3[:], op=mybir.AluOpType.mult
                    )
                if j == 0:
                    nc.sync.dma_start(gat_t[:], t_gat[:])
                # prefetch shared-path xT during the last slot's GEMM2
                if j == EPC - 1:
                    emit_xtt(0)
                    emit_xtt(1)
                # GEMM2 + gating scale -> ob [128, H] bf16, then scatter
                w2e = w2s[j]
                for rb in range(nblks[j]):
                    ob = obp.tile([128, 1, H], BF16, tag="ob")
                    for hc in range(H // 512):
                        p2 = ps.tile([128, 512], F32, tag="p2", bufs=3)
                        for f in range(FC):
                            nc.tensor.matmul(
                                p2[:],
                                hT[:, f, 128 * rb: 128 * (rb + 1)],
                                w2e[:, f, 512 * hc: 512 * (hc + 1)],
                                start=(f == 0), stop=(f == FC - 1),
                            )
                        nc.vector.tensor_scalar_mul(
                            ob[:, 0, 512 * hc: 512 * (hc + 1)], p2[:],
                            gat_t[:, boffs[j] + rb: boffs[j] + rb + 1],
                        )
                    nc.gpsimd.dma_scatter_add(
                        t_ymoe[:], ob[:],
                        idx_t[:, coffs[j] + 8 * rb: coffs[j] + 8 * (rb + 1)],
                        128, 128, H,
                    )

            # ---------------- shared expert (FS shard) ----------------
            sgt = shp.tile([128, HC, FSS], BF16, tag="sgt")
            sut = shp.tile([128, HC, FSS], BF16, tag="sut")
            sdt = shp.tile([128, FSS // 128, H], BF16, tag="sdt")
            for h2 in range(0, HC, 8):       # <=1.6us chunks
                nc.scalar.dma_start(sgt[:, h2: h2 + 8, :], t_sg[:, h2: h2 + 8, :])
                nc.scalar.dma_start(sut[:, h2: h2 + 8, :], t_su[:, h2: h2 + 8, :])
            for fs2 in range(FSS // 128):
                nc.scalar.dma_start(sdt[:, fs2: fs2 + 1, :], t_sd[:, fs2: fs2 + 1, :])
            for tb in range(N // 512):
                if tb + 2 < 4:
                    emit_xtt(tb + 2)
                xTt = xtts[tb]
                ttT = hp.tile([128, FSS // 128, 512], BF16, tag="ttT")
                for fs in range(FSS // 128):
                    pg = ps.tile([128, 512], F32, tag="p1")
                    pu = ps.tile([128, 512], F32, tag="p3")
                    for h in range(HC):
                        nc.tensor.matmul(
                            pg[:], sgt[:, h, 128 * fs: 128 * (fs + 1)],
                            xTt[:, h, :], start=(h == 0), stop=(h == HC - 1),
                        )
                    for h in range(HC):
                        nc.tensor.matmul(
                            pu[:], sut[:, h, 128 * fs: 128 * (fs + 1)],
                            xTt[:, h, :], start=(h == 0), stop=(h == HC - 1),
                        )
                    sig = hp.tile([128, 512], F32, tag="sigsh")
                    nc.scalar.activation(
                        sig[:], pg[:], mybir.ActivationFunctionType.Sigmoid
                    )
                    nc.vector.tensor_tensor(
                        sig[:], sig[:], pg[:], op=mybir.AluOpType.mult
                    )
                    nc.vector.tensor_tensor(
                        ttT[:, fs, :], sig[:], pu[:], op=mybir.AluOpType.mult
                    )
                for ts in range(4):
                    osh = obp.tile([128, 1, H], BF16, tag="ob")
                    for hc in range(H // 512):
                        p2 = ps.tile([128, 512], F32, tag="p2", bufs=3)
                        for fs in range(FSS // 128):
                            nc.tensor.matmul(
                                p2[:],
                                ttT[:, fs, 128 * ts: 128 * (ts + 1)],
                                sdt[:, fs, 512 * hc: 512 * (hc + 1)],
                                start=(fs == 0), stop=(fs == FSS // 128 - 1),
                            )
                        # split the PSUM drain across DVE and Act so the
                        # copy rate beats the GEMM produce rate
                        nc.vector.tensor_copy(
                            osh[:, 0, 512 * hc: 512 * hc + 256], p2[:, :256]
                        )
                        nc.scalar.activation(
                            osh[:, 0, 512 * hc + 256: 512 * (hc + 1)],
                            p2[:, 256:],
                            mybir.ActivationFunctionType.Copy,
                        )
                        # store each H-half as soon as its copies land
                        if hc % 2 == 1:
                            nc.sync.dma_start(
                                t_ysh[512 * tb + 128 * ts:
                                      512 * tb + 128 * (ts + 1),
                                      1024 * (hc // 2): 1024 * (hc // 2 + 1)],
                                osh[:, 0, 1024 * (hc // 2): 1024 * (hc // 2 + 1)],
                            )
    nc.compile()
    _CACHE[sched] = nc
    return nc


# ---------------------------------------------------------------- host glue
def _prep_inputs(hidden_states, gate_w, gate_bias, w1, w2, w3,
                 shared_gate, shared_up, shared_down):
    x = np.asarray(hidden_states, np.float32).reshape(N, H)

    def tile_lhsT(w):
        # [E, H, F] -> [E, FC, 128(h-part), HC*128]: tile (e,f)[p, hc*128+fo]
        # = w[e, 128*hc + p, 128*f + fo]
        we = w.reshape(-1, H // 128, 128, F // 128, 128)
        return np.ascontiguousarray(we.transpose(0, 3, 2, 1, 4)).reshape(
            -1, F // 128, 128, H)

    def tile_rhs(w, kc):
        # [E?, KC*128, M] -> [..., 128(part), KC, M]: (p, kc, m) = w[128*kc+p, m]
        wr = w.reshape(-1, kc, 128, w.shape[-1])
        return np.ascontiguousarray(wr.transpose(0, 2, 1, 3)).reshape(
            -1, 128, kc * w.shape[-1])

    topk_idx, topk_w = _gate_host_jax(x, np.asarray(gate_w, np.float32),
                                      np.asarray(gate_bias, np.float32))

    # capacity dispatch identical to reference: pos = per-expert running slot
    flat_e = topk_idx.reshape(-1)
    pos = np.zeros(N * K, np.int64)
    cnt = np.zeros(E, np.int64)
    for i, e in enumerate(flat_e):
        pos[i] = cnt[e]
        cnt[e] += 1
    keep = pos < CAP
    kept_counts = np.minimum(cnt, CAP)

    # Band schedule: sort experts by routed load (ascending, so the first
    # slot's gather is the smallest -> shortest startup), deal 8 per band
    # (one per core).  Static per-slot column cap = band max (padded to 4).
    order = np.argsort(kept_counts, kind="stable")
    scaps, nblks = [], []
    for j in range(EPC):
        cap = int(kept_counts[order[8 * j: 8 * (j + 1)]].max())
        cap = max(4, (cap + 3) // 4 * 4)
        scaps.append(cap)
        nblks.append((cap + 127) // 128)
    sched = (tuple(scaps), tuple(nblks))
    NBT = sum(nblks)
    SLOTS_TOT = 128 * NBT

    x_nat = np.zeros((N + 1, H), BF)
    x_nat[:N] = x.astype(BF)
    xb = x.astype(BF)
    xT = tile_rhs(xb.T, H // 128)[0].reshape(128, H // 128, N)
    w1b = tile_lhsT(np.asarray(w1, np.float32).astype(BF))
    w3b = tile_lhsT(np.asarray(w3, np.float32).astype(BF))
    w2b = tile_rhs(np.asarray(w2, np.float32).astype(BF), F // 128).reshape(
        E, 128, F // 128, H)
    sgb = np.asarray(shared_gate, np.float32).astype(BF)
    sub = np.asarray(shared_up, np.float32).astype(BF)
    sdb = np.asarray(shared_down, np.float32).astype(BF)
    tw_flat = topk_w.reshape(-1)

    in_maps = []
    for c in range(NCORES):
        idx = np.full(SLOTS_TOT, N, np.int16)  # pads -> dump row N
        gat = np.zeros(SLOTS_TOT, np.float32)  # pads -> weight 0
        experts = []
        off = 0
        for j in range(EPC):
            eg = int(order[8 * j + c])
            experts.append(eg)
            sel = np.nonzero((flat_e == eg) & keep)[0]
            idx[off: off + len(sel)] = sel // K
            gat[off: off + len(sel)] = tw_flat[sel]
            off += 128 * nblks[j]
        lo = c * FSS
        in_maps.append({
            "x_nat": x_nat,
            "xT": xT,
            "idx_w": _wrap_idx(idx),
            "gat_col": gat.reshape(NBT, 128).T.copy(),
            "w1": np.ascontiguousarray(w1b[experts]),
            "w3": np.ascontiguousarray(w3b[experts]),
            "w2": np.ascontiguousarray(w2b[experts]),
            "sg": tile_rhs(sgb[:, lo:lo + FSS], H // 128)[0].reshape(128, H // 128, FSS),
            "su": tile_rhs(sub[:, lo:lo + FSS], H // 128)[0].reshape(128, H // 128, FSS),
            "sd": tile_rhs(sdb[lo:lo + FSS, :], FSS // 128)[0].reshape(128, FSS // 128, H),
        })
    return in_maps, sched


def kernel(hidden_states, gate_w, gate_bias, w1, w2, w3,
           shared_gate, shared_up, shared_down, _trace=False):
    in_maps, sched = _prep_inputs(hidden_states, gate_w, gate_bias, w1, w2, w3,
                                  shared_gate, shared_up, shared_down)
    nc = _build(sched)
    res = run_bass_kernel_spmd(nc, in_maps, list(range(NCORES)), trace=_trace)
    y = np.zeros((N, H), np.float64)
    for r in res.results:
        y += r["y_moe"][:N].astype(np.float64)
        y += r["y_sh"].astype(np.float64)
    out = y.astype(np.float32).reshape(B, S, H)
    if _trace:
        kernel._last = res
    return out


# revision 24
# speedup vs baseline: 1.0092x; 1.0030x over previous
"""KimiSparseMoeBlock kernel for 8 Trainium2 NeuronCores.

Sharding (expert-parallel, per spec hint):
  - 32 experts sharded 4-per-core (w1/w2/w3 leading dim), with the
    expert->core assignment chosen at runtime from the actual routed
    load: experts are sorted by token count and dealt into 4 "bands"
    (8 experts each, one per core).  Band j's static column capacity is
    the max count within the band, so the compiled SPMD program only
    computes (close to) the real routed tokens instead of the full
    2x-mean capacity buffer the reference uses.
  - Shared SwiGLU tensor-parallel along FS (2048/8 = 256 per core).
  - Token dispatch/combine done ON DEVICE via dma_gather(transpose) /
    dma_scatter_add with per-core index lists; gate/top-k routing
    metadata is computed host-side during input sharding (~0.1% FLOPs).
  - Each core emits two partial outputs (scatter-accumulated MoE rows
    and the FS-shard of the shared expert); unshard = sum of partials.

Device pipeline per core (per expert slot j with column cap S_j):
  gather ceil(S_j/128)*128 token rows (bf16, transposed to [H, W]) ->
  SwiGLU GEMMs on exactly S_j columns (bf16 PE, fp32 PSUM) ->
  gating-scaled rows -> dma_scatter_add into y_moe; plus FS-sharded
  shared SwiGLU from host-pretransposed xT.
"""
import os
import sys

sys.path.insert(0, "/opt/trn_rl_repo")

import numpy as np
import ml_dtypes

import concourse.bass as bass
import concourse.bacc as bacc
import concourse.tile as tile
import concourse.mybir as mybir
from concourse.bass_utils import run_bass_kernel_spmd

F32 = mybir.dt.float32
BF16 = mybir.dt.bfloat16
I16 = mybir.dt.int16
BF = ml_dtypes.bfloat16

E, K, G, TG = 32, 4, 4, 2
H, F, FS = 2048, 1024, 2048
SCALE = 2.0
B, S = 2, 1024
N = B * S
CAP = 2 * (N * K // E)  # 512 (reference capacity; tokens beyond it drop)
NCORES = 8
EPC = E // NCORES       # expert slots per core = 4
FSS = FS // NCORES      # shared intermediate per core = 256
HC = H // 128           # 16 contraction chunks
FC = F // 128           # 8 F chunks


# ---------------------------------------------------------------- routing
def _gate_host(x, gate_w, gate_bias):
    """Replicate reference _gate in numpy (f32, jax-compatible ops)."""
    x = x.astype(np.float32)
    logits = x @ gate_w.T.astype(np.float32)
    scores = 1.0 / (1.0 + np.exp(-logits))
    sc = scores + gate_bias[None, :]
    n = x.shape[0]
    grp = sc.reshape(n, G, E // G)
    top2 = np.sort(grp, axis=-1)[:, :, -2:]
    group_scores = top2.sum(-1)
    gidx = np.argsort(-group_scores, axis=-1, kind="stable")[:, :TG]
    gmask = np.zeros((n, G), np.float32)
    np.put_along_axis(gmask, gidx, 1.0, axis=1)
    smask = np.repeat(gmask, E // G, axis=1)
    tmp = np.where(smask > 0, sc, 0.0)
    topk_idx = np.argsort(-tmp, axis=-1, kind="stable")[:, :K].astype(np.int32)
    topk_w = np.take_along_axis(scores, topk_idx, axis=1)
    topk_w = topk_w / (topk_w.sum(-1, keepdims=True) + 1e-20)
    return topk_idx, (topk_w * SCALE).astype(np.float32)


def _gate_host_jax(x, gate_w, gate_bias):
    """Bit-exact replication of the reference gate on jax-cpu."""
    try:
        import jax

        import jax.numpy as jnp

        def gate(x, gate_w, gate_bias):
            logits = x @ gate_w.T
            scores = jax.nn.sigmoid(logits)
            sc = scores + gate_bias
            n = x.shape[0]
            grp = sc.reshape(n, G, E // G)
            group_scores = jax.lax.top_k(grp, 2)[0].sum(-1)
            _, gidx = jax.lax.top_k(group_scores, TG)
            gmask = (
                jnp.zeros((n, G), sc.dtype)
                .at[jnp.arange(n)[:, None], gidx]
                .set(1.0)
            )
            smask = jnp.repeat(gmask, E // G, axis=1)
            tmp = jnp.where(smask > 0, sc, 0.0)
            _, topk_idx = jax.lax.top_k(tmp, K)
            topk_w = jnp.take_along_axis(scores, topk_idx, axis=1)
            topk_w = topk_w / (topk_w.sum(-1, keepdims=True) + 1e-20)
            return topk_idx, topk_w * SCALE

        with jax.default_device(jax.devices("cpu")[0]):
            ti, tw = jax.jit(gate, backend="cpu")(x, gate_w, gate_bias)
        return np.asarray(ti, np.int32), np.asarray(tw, np.float32)
    except Exception:
        return _gate_host(x, gate_w, gate_bias)


def _wrap_idx(idx):
    """[n] -> [128, n//16]: slot s at partition s%16 (replicated x8), col s//16."""
    n = idx.shape[0]
    w = idx.reshape(n // 16, 16).T
    return np.tile(w, (8, 1)).copy()


# ---------------------------------------------------------------- bass build
_CACHE = {}


def _build(sched):
    """sched = (scaps, nblks): static per-slot column caps / 128-blocks."""
    if sched in _CACHE:
        return _CACHE[sched]
    scaps, nblks = sched
    NBT = sum(nblks)           # total 128-row token blocks per core
    SLOTS_TOT = 128 * NBT

    nc = bacc.Bacc("TRN2", target_bir_lowering=False, debug=False,
                   num_devices=NCORES)
    t_x = nc.dram_tensor("x_nat", [N + 1, H], BF16, kind="ExternalInput")
    t_xT = nc.dram_tensor("xT", [128, H // 128, N], BF16, kind="ExternalInput")
    t_idx = nc.dram_tensor("idx_w", [128, SLOTS_TOT // 16], I16,
                           kind="ExternalInput")
    t_gat = nc.dram_tensor("gat_col", [128, NBT], F32, kind="ExternalInput")
    t_w1 = nc.dram_tensor("w1", [EPC, F // 128, 128, H], BF16, kind="ExternalInput")
    t_w3 = nc.dram_tensor("w3", [EPC, F // 128, 128, H], BF16, kind="ExternalInput")
    t_w2 = nc.dram_tensor("w2", [EPC, 128, F // 128, H], BF16, kind="ExternalInput")
    t_sg = nc.dram_tensor("sg", [128, H // 128, FSS], BF16, kind="ExternalInput")
    t_su = nc.dram_tensor("su", [128, H // 128, FSS], BF16, kind="ExternalInput")
    t_sd = nc.dram_tensor("sd", [128, FSS // 128, H], BF16, kind="ExternalInput")
    t_ymoe = nc.dram_tensor("y_moe", [N + 1, H], BF16, kind="ExternalOutput")
    t_ysh = nc.dram_tensor("y_sh", [N, H], BF16, kind="ExternalOutput")

    coffs = np.cumsum([0] + [8 * nb for nb in nblks]).tolist()   # idx col offs
    boffs = np.cumsum([0] + list(nblks)).tolist()                # block offs

    with tile.TileContext(nc) as tc:
        with (
            tc.tile_pool(name="idxp", bufs=1) as idxp,
            tc.tile_pool(name="gath", bufs=2) as gath,
            tc.tile_pool(name="wp", bufs=3) as wp,
            tc.tile_pool(name="w2p", bufs=2) as w2p,
            tc.tile_pool(name="hp", bufs=2) as hp,
            tc.tile_pool(name="obp", bufs=2) as obp,
            tc.tile_pool(name="shp", bufs=1) as shp,
            tc.tile_pool(name="ps", bufs=2, space="PSUM") as ps,
        ):
            # idx on the pool queue (its consumer): the first gather starts
            # as soon as it lands instead of queueing behind w2 chunks.
            idx_t = idxp.tile([128, SLOTS_TOT // 16], I16)
            nc.gpsimd.dma_start(idx_t[:], t_idx[:])
            gat_t = idxp.tile([128, NBT], F32)   # load deferred (needed ~30us in)

            bufs = [None] * EPC   # gathered token tiles (list per slot)
            w2s = [None] * EPC    # w2 weight tiles
            xtts = [None] * 4     # shared-path xT tiles

            def emit_gather(j, split=1):
                # split>1: issue the gather as `split` pieces so the first
                # GEMM can start before the whole slot is resident.
                W = 128 * nblks[j]
                tiles = []
                step = W // split
                assert step % 128 == 0
                for s in range(split):
                    t = gath.tile([128, HC, step], BF16, tag="bufT",
                                  name=f"bufT{j}_{s}")
                    c0 = coffs[j] + s * step // 16
                    nc.gpsimd.dma_gather(
                        t[:], t_x[:], idx_t[:, c0: c0 + step // 16],
                        step, step, H, transpose=True,
                    )
                    tiles.append(t)
                bufs[j] = tiles

            def emit_w2(j, split=1):
                # chunked per-f so urgent ops never queue behind a 4MB DMA;
                # split=2 halves the chunks again (startup-critical window)
                w2s[j] = w2p.tile([128, FC, H], BF16, tag="w2e",
                                  name=f"w2e{j}")
                hstep = H // split
                for f in range(FC):
                    for s in range(split):
                        nc.gpsimd.dma_start(
                            w2s[j][:, f, hstep * s: hstep * (s + 1)],
                            t_w2[j, :, f, hstep * s: hstep * (s + 1)],
                        )

            def emit_xtt(tb):
                xtts[tb] = gath.tile([128, HC, 512], BF16, tag="xTt",
                                     name=f"xTt{tb}")
                for s in range(4):
                    nc.sync.dma_start(
                        xtts[tb][:, :, 128 * s: 128 * (s + 1)],
                        t_xT[:, :, 512 * tb + 128 * s: 512 * tb + 128 * (s + 1)],
                    )

            # Prefetch: slot 0/1 gathers + slot 0/1 w2 up-front (pool queue);
            # slot j+2's gather/w2 are emitted at slot j+1's head so they sit
            # behind slot j's scatters but well ahead of their consumers.
            with tc.high_priority():
                emit_gather(0, split=2 if nblks[0] >= 2 else 1)
                emit_gather(1)
            # hold the w2 prefetch off the pool queue until the first
            # gather (the PE-critical op) has had its chance to run
            with tc.tile_wait_until(0.0025):
                emit_w2(0, split=2)
            emit_w2(1)

            # ---------------- expert path ----------------
            for j in range(EPC):
                Sj = scaps[j]
                W = 128 * nblks[j]
                if j >= 1 and j + 1 < EPC:
                    emit_gather(j + 1)
                    emit_w2(j + 1)
                # clip gather pieces to the Sj real columns
                pieces = []
                off = 0
                for t in bufs[j]:
                    w = t.shape[2]
                    lo = off
                    hi = min(off + w, Sj)
                    if hi > lo:
                        pieces.append((t, lo, hi - lo))
                    off += w
                # GEMM1 + GEMM3 + SwiGLU -> hT [128, FC, W] bf16
                hT = hp.tile([128, FC, W], BF16, tag="hT", name=f"hT{j}")
                if Sj < W:
                    nc.vector.memset(hT[:, :, Sj:], 0.0)
                for f in range(FC):
                    w1f = wp.tile([128, HC, 128], BF16, tag="w1f")
                    w3f = wp.tile([128, HC, 128], BF16, tag="w3f")
                    # first w1 of the run goes via SP — the Act queue opens
                    # with a 1.3us act-table load
                    w1eng = nc.sync if (j == 0 and f == 0) else nc.scalar
                    w1eng.dma_start(
                        w1f[:], t_w1[j, f].rearrange("p (hc fo) -> p hc fo", hc=HC)
                    )
                    nc.sync.dma_start(
                        w3f[:], t_w3[j, f].rearrange("p (hc fo) -> p hc fo", hc=HC)
                    )
                    p1 = ps.tile([128, Sj], F32, tag="p1")
                    p3 = ps.tile([128, Sj], F32, tag="p3")
                    for (t, lo, n) in pieces:
                        for h in range(HC):
                            nc.tensor.matmul(p1[:, lo: lo + n], w1f[:, h, :],
                                             t[:, h, :n],
                                             start=(h == 0), stop=(h == HC - 1))
                    for (t, lo, n) in pieces:
                        for h in range(HC):
                            nc.tensor.matmul(p3[:, lo: lo + n], w3f[:, h, :],
                                             t[:, h, :n],
                                             start=(h == 0), stop=(h == HC - 1))
                    sig = hp.tile([128, Sj], F32, tag="sig")
                    nc.scalar.activation(
                        sig[:], p1[:], mybir.ActivationFunctionType.Sigmoid
                    )
                    nc.vector.tensor_tensor(
                        sig[:], sig[:], p1[:], op=mybir.AluOpType.mult
                    )
                    nc.vector.tensor_tensor(
                        hT[:, f, :Sj], sig[:], p3[:], op=mybir.AluOpType.mult
                    )
                if j == 0:
                    nc.sync.dma_start(gat_t[:], t_gat[:])
                # prefetch shared-path xT during the last slot's GEMM2
                if j == EPC - 1:
                    emit_xtt(0)
                    emit_xtt(1)
                # GEMM2 + gating scale -> ob [128, H] bf16, then scatter
                w2e = w2s[j]
                for rb in range(nblks[j]):
                    ob = obp.tile([128, 1, H], BF16, tag="ob")
                    for hc in range(H // 512):
                        p2 = ps.tile([128, 512], F32, tag="p2", bufs=3)
                        for f in range(FC):
                            nc.tensor.matmul(
                                p2[:],
                                hT[:, f, 128 * rb: 128 * (rb + 1)],
                                w2e[:, f, 512 * hc: 512 * (hc + 1)],
                                start=(f == 0), stop=(f == FC - 1),
                            )
                        nc.vector.tensor_scalar_mul(
                            ob[:, 0, 512 * hc: 512 * (hc + 1)], p2[:],
                            gat_t[:, boffs[j] + rb: boffs[j] + rb + 1],
                        )
                    nc.gpsimd.dma_scatter_add(
                        t_ymoe[:], ob[:],
                        idx_t[:, coffs[j] + 8 * rb: coffs[j] + 8 * (rb + 1)],
                        128, 128, H,
                    )

            # ---------------- shared expert (FS shard) ----------------
            sgt = shp.tile([128, HC, FSS], BF16, tag="sgt")
            sut = shp.tile([128, HC, FSS], BF16, tag="sut")
            sdt = shp.tile([128, FSS // 128, H], BF16, tag="sdt")
            for h2 in range(0, HC, 8):       # <=1.6us chunks
                nc.scalar.dma_start(sgt[:, h2: h2 + 8, :], t_sg[:, h2: h2 + 8, :])
                nc.scalar.dma_start(sut[:, h2: h2 + 8, :], t_su[:, h2: h2 + 8, :])
            for fs2 in range(FSS // 128):
                nc.scalar.dma_start(sdt[:, fs2: fs2 + 1, :], t_sd[:, fs2: fs2 + 1, :])
            for tb in range(N // 512):
                if tb + 2 < 4:
                    emit_xtt(tb + 2)
                xTt = xtts[tb]
                ttT = hp.tile([128, FSS // 128, 512], BF16, tag="ttT")
                for fs in range(FSS // 128):
                    pg = ps.tile([128, 512], F32, tag="p1")
                    pu = ps.tile([128, 512], F32, tag="p3")
                    for h in range(HC):
                        nc.tensor.matmul(
                            pg[:], sgt[:, h, 128 * fs: 128 * (fs + 1)],
                            xTt[:, h, :], start=(h == 0), stop=(h == HC - 1),
                        )
                    for h in range(HC):
                        nc.tensor.matmul(
                            pu[:], sut[:, h, 128 * fs: 128 * (fs + 1)],
                            xTt[:, h, :], start=(h == 0), stop=(h == HC - 1),
                        )
                    sig = hp.tile([128, 512], F32, tag="sigsh")
                    nc.scalar.activation(
                        sig[:], pg[:], mybir.ActivationFunctionType.Sigmoid
                    )
                    nc.vector.tensor_tensor(
                        sig[:], sig[:], pg[:], op=mybir.AluOpType.mult
                    )
                    nc.vector.tensor_tensor(
                        ttT[:, fs, :], sig[:], pu[:], op=mybir.AluOpType.mult
                    )
                for ts in range(4):
                    osh = obp.tile([128, 1, H], BF16, tag="ob")
                    for hc in range(H // 512):
                        p2 = ps.tile([128, 512], F32, tag="p2", bufs=3)
                        for fs in range(FSS // 128):
                            nc.tensor.matmul(
                                p2[:],
                                ttT[:, fs, 128 * ts: 128 * (ts + 1)],
                                sdt[:, fs, 512 * hc: 512 * (hc + 1)],
                                start=(fs == 0), stop=(fs == FSS // 128 - 1),
                            )
                        # split the PSUM drain across DVE and Act so the
                        # copy rate beats the GEMM produce rate
                        nc.vector.tensor_copy(
                            osh[:, 0, 512 * hc: 512 * hc + 256], p2[:, :256]
                        )
                        nc.scalar.activation(
                            osh[:, 0, 512 * hc + 256: 512 * (hc + 1)],
                            p2[:, 256:],
                            mybir.ActivationFunctionType.Copy,
                        )
                        # store each H-half as soon as its copies land
                        if hc % 2 == 1:
                            nc.sync.dma_start(
                                t_ysh[512 * tb + 128 * ts:
                                      512 * tb + 128 * (ts + 1),
                                      1024 * (hc // 2): 1024 * (hc // 2 + 1)],
                                osh[:, 0, 1024 * (hc // 2): 1024 * (hc // 2 + 1)],
                            )
    nc.compile()
    _CACHE[sched] = nc
    return nc


# ---------------------------------------------------------------- host glue
def _prep_inputs(hidden_states, gate_w, gate_bias, w1, w2, w3,
                 shared_gate, shared_up, shared_down):
    x = np.asarray(hidden_states, np.float32).reshape(N, H)

    def tile_lhsT(w):
        # [E, H, F] -> [E, FC, 128(h-part), HC*128]: tile (e,f)[p, hc*128+fo]
        # = w[e, 128*hc + p, 128*f + fo]
        we = w.reshape(-1, H // 128, 128, F // 128, 128)
        return np.ascontiguousarray(we.transpose(0, 3, 2, 1, 4)).reshape(
            -1, F // 128, 128, H)

    def tile_rhs(w, kc):
        # [E?, KC*128, M] -> [..., 128(part), KC, M]: (p, kc, m) = w[128*kc+p, m]
        wr = w.reshape(-1, kc, 128, w.shape[-1])
        return np.ascontiguousarray(wr.transpose(0, 2, 1, 3)).reshape(
            -1, 128, kc * w.shape[-1])

    topk_idx, topk_w = _gate_host_jax(x, np.asarray(gate_w, np.float32),
                                      np.asarray(gate_bias, np.float32))

    # capacity dispatch identical to reference: pos = per-expert running slot
    flat_e = topk_idx.reshape(-1)
    pos = np.zeros(N * K, np.int64)
    cnt = np.zeros(E, np.int64)
    for i, e in enumerate(flat_e):
        pos[i] = cnt[e]
        cnt[e] += 1
    keep = pos < CAP
    kept_counts = np.minimum(cnt, CAP)

    # Band schedule: sort experts by routed load (ascending, so the first
    # slot's gather is the smallest -> shortest startup), deal 8 per band
    # (one per core).  Static per-slot column cap = band max (padded to 4).
    order = np.argsort(kept_counts, kind="stable")
    scaps, nblks = [], []
    for j in range(EPC):
        cap = max(1, int(kept_counts[order[8 * j: 8 * (j + 1)]].max()))
        scaps.append(cap)
        nblks.append((cap + 127) // 128)
    sched = (tuple(scaps), tuple(nblks))
    NBT = sum(nblks)
    SLOTS_TOT = 128 * NBT

    x_nat = np.zeros((N + 1, H), BF)
    x_nat[:N] = x.astype(BF)
    xb = x.astype(BF)
    xT = tile_rhs(xb.T, H // 128)[0].reshape(128, H // 128, N)
    w1b = tile_lhsT(np.asarray(w1, np.float32).astype(BF))
    w3b = tile_lhsT(np.asarray(w3, np.float32).astype(BF))
    w2b = tile_rhs(np.asarray(w2, np.float32).astype(BF), F // 128).reshape(
        E, 128, F // 128, H)
    sgb = np.asarray(shared_gate, np.float32).astype(BF)
    sub = np.asarray(shared_up, np.float32).astype(BF)
    sdb = np.asarray(shared_down, np.float32).astype(BF)
    tw_flat = topk_w.reshape(-1)

    in_maps = []
    for c in range(NCORES):
        idx = np.full(SLOTS_TOT, N, np.int16)  # pads -> dump row N
        gat = np.zeros(SLOTS_TOT, np.float32)  # pads -> weight 0
        experts = []
        off = 0
        for j in range(EPC):
            eg = int(order[8 * j + c])
            experts.append(eg)
            sel = np.nonzero((flat_e == eg) & keep)[0]
            idx[off: off + len(sel)] = sel // K
            gat[off: off + len(sel)] = tw_flat[sel]
            off += 128 * nblks[j]
        lo = c * FSS
        in_maps.append({
            "x_nat": x_nat,
            "xT": xT,
            "idx_w": _wrap_idx(idx),
            "gat_col": gat.reshape(NBT, 128).T.copy(),
            "w1": np.ascontiguousarray(w1b[experts]),
            "w3": np.ascontiguousarray(w3b[experts]),
            "w2": np.ascontiguousarray(w2b[experts]),
            "sg": tile_rhs(sgb[:, lo:lo + FSS], H // 128)[0].reshape(128, H // 128, FSS),
            "su": tile_rhs(sub[:, lo:lo + FSS], H // 128)[0].reshape(128, H // 128, FSS),
            "sd": tile_rhs(sdb[lo:lo + FSS, :], FSS // 128)[0].reshape(128, FSS // 128, H),
        })
    return in_maps, sched


def kernel(hidden_states, gate_w, gate_bias, w1, w2, w3,
           shared_gate, shared_up, shared_down, _trace=False):
    in_maps, sched = _prep_inputs(hidden_states, gate_w, gate_bias, w1, w2, w3,
                                  shared_gate, shared_up, shared_down)
    nc = _build(sched)
    res = run_bass_kernel_spmd(nc, in_maps, list(range(NCORES)), trace=_trace)
    y = np.zeros((N, H), np.float64)
    for r in res.results:
        y += r["y_moe"][:N].astype(np.float64)
        y += r["y_sh"].astype(np.float64)
    out = y.astype(np.float32).reshape(B, S, H)
    if _trace:
        kernel._last = res
    return out


# revision 26
# speedup vs baseline: 1.0101x; 1.0009x over previous
"""KimiSparseMoeBlock kernel for 8 Trainium2 NeuronCores.

Sharding (expert-parallel, per spec hint):
  - 32 experts sharded 4-per-core (w1/w2/w3 leading dim), with the
    expert->core assignment chosen at runtime from the actual routed
    load: experts are sorted by token count and dealt into 4 "bands"
    (8 experts each, one per core).  Band j's static column capacity is
    the max count within the band, so the compiled SPMD program only
    computes (close to) the real routed tokens instead of the full
    2x-mean capacity buffer the reference uses.
  - Shared SwiGLU tensor-parallel along FS (2048/8 = 256 per core).
  - Token dispatch/combine done ON DEVICE via dma_gather(transpose) /
    dma_scatter_add with per-core index lists; gate/top-k routing
    metadata is computed host-side during input sharding (~0.1% FLOPs).
  - Each core emits two partial outputs (scatter-accumulated MoE rows
    and the FS-shard of the shared expert); unshard = sum of partials.

Device pipeline per core (per expert slot j with column cap S_j):
  gather ceil(S_j/128)*128 token rows (bf16, transposed to [H, W]) ->
  SwiGLU GEMMs on exactly S_j columns (bf16 PE, fp32 PSUM) ->
  gating-scaled rows -> dma_scatter_add into y_moe; plus FS-sharded
  shared SwiGLU from host-pretransposed xT.
"""
import os
import sys

sys.path.insert(0, "/opt/trn_rl_repo")

import numpy as np
import ml_dtypes

import concourse.bass as bass
import concourse.bacc as bacc
import concourse.tile as tile
import concourse.mybir as mybir
from concourse.bass_utils import run_bass_kernel_spmd

F32 = mybir.dt.float32
BF16 = mybir.dt.bfloat16
I16 = mybir.dt.int16
BF = ml_dtypes.bfloat16

E, K, G, TG = 32, 4, 4, 2
H, F, FS = 2048, 1024, 2048
SCALE = 2.0
B, S = 2, 1024
N = B * S
CAP = 2 * (N * K // E)  # 512 (reference capacity; tokens beyond it drop)
NCORES = 8
EPC = E // NCORES       # expert slots per core = 4
FSS = FS // NCORES      # shared intermediate per core = 256
HC = H // 128           # 16 contraction chunks
FC = F // 128           # 8 F chunks


# ---------------------------------------------------------------- routing
def _gate_host(x, gate_w, gate_bias):
    """Replicate reference _gate in numpy (f32, jax-compatible ops)."""
    x = x.astype(np.float32)
    logits = x @ gate_w.T.astype(np.float32)
    scores = 1.0 / (1.0 + np.exp(-logits))
    sc = scores + gate_bias[None, :]
    n = x.shape[0]
    grp = sc.reshape(n, G, E // G)
    top2 = np.sort(grp, axis=-1)[:, :, -2:]
    group_scores = top2.sum(-1)
    gidx = np.argsort(-group_scores, axis=-1, kind="stable")[:, :TG]
    gmask = np.zeros((n, G), np.float32)
    np.put_along_axis(gmask, gidx, 1.0, axis=1)
    smask = np.repeat(gmask, E // G, axis=1)
    tmp = np.where(smask > 0, sc, 0.0)
    topk_idx = np.argsort(-tmp, axis=-1, kind="stable")[:, :K].astype(np.int32)
    topk_w = np.take_along_axis(scores, topk_idx, axis=1)
    topk_w = topk_w / (topk_w.sum(-1, keepdims=True) + 1e-20)
    return topk_idx, (topk_w * SCALE).astype(np.float32)


def _gate_host_jax(x, gate_w, gate_bias):
    """Bit-exact replication of the reference gate on jax-cpu."""
    try:
        import jax

        import jax.numpy as jnp

        def gate(x, gate_w, gate_bias):
            logits = x @ gate_w.T
            scores = jax.nn.sigmoid(logits)
            sc = scores + gate_bias
            n = x.shape[0]
            grp = sc.reshape(n, G, E // G)
            group_scores = jax.lax.top_k(grp, 2)[0].sum(-1)
            _, gidx = jax.lax.top_k(group_scores, TG)
            gmask = (
                jnp.zeros((n, G), sc.dtype)
                .at[jnp.arange(n)[:, None], gidx]
                .set(1.0)
            )
            smask = jnp.repeat(gmask, E // G, axis=1)
            tmp = jnp.where(smask > 0, sc, 0.0)
            _, topk_idx = jax.lax.top_k(tmp, K)
            topk_w = jnp.take_along_axis(scores, topk_idx, axis=1)
            topk_w = topk_w / (topk_w.sum(-1, keepdims=True) + 1e-20)
            return topk_idx, topk_w * SCALE

        with jax.default_device(jax.devices("cpu")[0]):
            ti, tw = jax.jit(gate, backend="cpu")(x, gate_w, gate_bias)
        return np.asarray(ti, np.int32), np.asarray(tw, np.float32)
    except Exception:
        return _gate_host(x, gate_w, gate_bias)


def _wrap_idx(idx):
    """[n] -> [128, n//16]: slot s at partition s%16 (replicated x8), col s//16."""
    n = idx.shape[0]
    w = idx.reshape(n // 16, 16).T
    return np.tile(w, (8, 1)).copy()


# ---------------------------------------------------------------- bass build
_CACHE = {}


def _build(sched):
    """sched = (scaps, nblks): static per-slot column caps / 128-blocks."""
    if sched in _CACHE:
        return _CACHE[sched]
    scaps, nblks = sched
    NBT = sum(nblks)           # total 128-row token blocks per core
    SLOTS_TOT = 128 * NBT

    nc = bacc.Bacc("TRN2", target_bir_lowering=False, debug=False,
                   num_devices=NCORES)
    t_x = nc.dram_tensor("x_nat", [N + 1, H], BF16, kind="ExternalInput")
    t_xT = nc.dram_tensor("xT", [128, H // 128, N], BF16, kind="ExternalInput")
    t_idx = nc.dram_tensor("idx_w", [128, SLOTS_TOT // 16], I16,
                           kind="ExternalInput")
    t_gat = nc.dram_tensor("gat_col", [128, NBT], F32, kind="ExternalInput")
    t_w1 = nc.dram_tensor("w1", [EPC, F // 128, 128, H], BF16, kind="ExternalInput")
    t_w3 = nc.dram_tensor("w3", [EPC, F // 128, 128, H], BF16, kind="ExternalInput")
    t_w2 = nc.dram_tensor("w2", [EPC, 128, F // 128, H], BF16, kind="ExternalInput")
    t_sg = nc.dram_tensor("sg", [128, H // 128, FSS], BF16, kind="ExternalInput")
    t_su = nc.dram_tensor("su", [128, H // 128, FSS], BF16, kind="ExternalInput")
    t_sd = nc.dram_tensor("sd", [128, FSS // 128, H], BF16, kind="ExternalInput")
    t_ymoe = nc.dram_tensor("y_moe", [N + 1, H], BF16, kind="ExternalOutput")
    t_ysh = nc.dram_tensor("y_sh", [N, H], BF16, kind="ExternalOutput")

    coffs = np.cumsum([0] + [8 * nb for nb in nblks]).tolist()   # idx col offs
    boffs = np.cumsum([0] + list(nblks)).tolist()                # block offs

    with tile.TileContext(nc) as tc:
        with (
            tc.tile_pool(name="idxp", bufs=1) as idxp,
            tc.tile_pool(name="gath", bufs=2) as gath,
            tc.tile_pool(name="wp", bufs=3) as wp,
            tc.tile_pool(name="w2p", bufs=2) as w2p,
            tc.tile_pool(name="hp", bufs=2) as hp,
            tc.tile_pool(name="obp", bufs=2) as obp,
            tc.tile_pool(name="shp", bufs=1) as shp,
            tc.tile_pool(name="ps", bufs=2, space="PSUM") as ps,
        ):
            # idx on the pool queue (its consumer): the first gather starts
            # as soon as it lands instead of queueing behind w2 chunks.
            idx_t = idxp.tile([128, SLOTS_TOT // 16], I16)
            nc.gpsimd.dma_start(idx_t[:], t_idx[:])
            gat_t = idxp.tile([128, NBT], F32)   # load deferred (needed ~30us in)

            # warm the sigmoid act-table during an Act-queue idle window so
            # the 1.3us table load doesn't land mid-pipeline at first use
            with tc.tile_wait_until(0.005):
                warm = idxp.tile([128, 1], F32)
                nc.vector.memset(warm[:], 0.0)
                warm2 = idxp.tile([128, 1], F32)
                nc.scalar.activation(warm2[:], warm[:],
                                     mybir.ActivationFunctionType.Sigmoid)

            bufs = [None] * EPC   # gathered token tiles (list per slot)
            w2s = [None] * EPC    # w2 weight tiles
            xtts = [None] * 4     # shared-path xT tiles

            def emit_gather(j, split=1):
                # split>1: issue the gather as `split` pieces so the first
                # GEMM can start before the whole slot is resident.
                W = 128 * nblks[j]
                tiles = []
                step = W // split
                assert step % 128 == 0
                for s in range(split):
                    t = gath.tile([128, HC, step], BF16, tag="bufT",
                                  name=f"bufT{j}_{s}")
                    c0 = coffs[j] + s * step // 16
                    nc.gpsimd.dma_gather(
                        t[:], t_x[:], idx_t[:, c0: c0 + step // 16],
                        step, step, H, transpose=True,
                    )
                    tiles.append(t)
                bufs[j] = tiles

            def emit_w2(j, split=1):
                # chunked per-f so urgent ops never queue behind a 4MB DMA;
                # split=2 halves the chunks again (startup-critical window)
                w2s[j] = w2p.tile([128, FC, H], BF16, tag="w2e",
                                  name=f"w2e{j}")
                hstep = H // split
                for f in range(FC):
                    for s in range(split):
                        nc.gpsimd.dma_start(
                            w2s[j][:, f, hstep * s: hstep * (s + 1)],
                            t_w2[j, :, f, hstep * s: hstep * (s + 1)],
                        )

            def emit_xtt(tb):
                # shares the bufT tag: expert gathers are dead by the time
                # each xT tile loads, so the same two buffers cycle on
                xtts[tb] = gath.tile([128, HC, 512], BF16, tag="bufT",
                                     name=f"xTt{tb}")
                for s in range(4):
                    nc.sync.dma_start(
                        xtts[tb][:, :, 128 * s: 128 * (s + 1)],
                        t_xT[:, :, 512 * tb + 128 * s: 512 * tb + 128 * (s + 1)],
                    )

            # Prefetch: slot 0/1 gathers + slot 0/1 w2 up-front (pool queue);
            # slot j+2's gather/w2 are emitted at slot j+1's head so they sit
            # behind slot j's scatters but well ahead of their consumers.
            with tc.high_priority():
                emit_gather(0, split=2 if nblks[0] >= 2 else 1)
                emit_gather(1)
            # hold the w2 prefetch off the pool queue until the first
            # gather (the PE-critical op) has had its chance to run
            with tc.tile_wait_until(0.0025):
                emit_w2(0, split=2)
            emit_w2(1)

            # ---------------- expert path ----------------
            for j in range(EPC):
                Sj = scaps[j]
                W = 128 * nblks[j]
                if j >= 1 and j + 1 < EPC:
                    emit_gather(j + 1)
                    emit_w2(j + 1)
                # clip gather pieces to the Sj real columns
                pieces = []
                off = 0
                for t in bufs[j]:
                    w = t.shape[2]
                    lo = off
                    hi = min(off + w, Sj)
                    if hi > lo:
                        pieces.append((t, lo, hi - lo))
                    off += w
                # GEMM1 + GEMM3 + SwiGLU -> hT [128, FC, W] bf16
                hT = hp.tile([128, FC, W], BF16, tag="hT", name=f"hT{j}")
                if Sj < W:
                    nc.vector.memset(hT[:, :, Sj:], 0.0)
                for f in range(FC):
                    w1f = wp.tile([128, HC, 128], BF16, tag="w1f")
                    w3f = wp.tile([128, HC, 128], BF16, tag="w3f")
                    # first w1 of the run goes via SP — the Act queue opens
                    # with a 1.3us act-table load
                    w1eng = nc.sync if (j == 0 and f == 0) else nc.scalar
                    w1eng.dma_start(
                        w1f[:], t_w1[j, f].rearrange("p (hc fo) -> p hc fo", hc=HC)
                    )
                    nc.sync.dma_start(
                        w3f[:], t_w3[j, f].rearrange("p (hc fo) -> p hc fo", hc=HC)
                    )
                    p1 = ps.tile([128, Sj], F32, tag="p1")
                    p3 = ps.tile([128, Sj], F32, tag="p3")
                    for (t, lo, n) in pieces:
                        for h in range(HC):
                            nc.tensor.matmul(p1[:, lo: lo + n], w1f[:, h, :],
                                             t[:, h, :n],
                                             start=(h == 0), stop=(h == HC - 1))
                    for (t, lo, n) in pieces:
                        for h in range(HC):
                            nc.tensor.matmul(p3[:, lo: lo + n], w3f[:, h, :],
                                             t[:, h, :n],
                                             start=(h == 0), stop=(h == HC - 1))
                    sig = hp.tile([128, Sj], F32, tag="sig")
                    nc.scalar.activation(
                        sig[:], p1[:], mybir.ActivationFunctionType.Sigmoid
                    )
                    nc.vector.tensor_tensor(
                        sig[:], sig[:], p1[:], op=mybir.AluOpType.mult
                    )
                    nc.vector.tensor_tensor(
                        hT[:, f, :Sj], sig[:], p3[:], op=mybir.AluOpType.mult
                    )
                if j == 0:
                    nc.sync.dma_start(gat_t[:], t_gat[:])
                # prefetch shared-path xT during the last slot's GEMM2
                if j == EPC - 1:
                    emit_xtt(0)
                    emit_xtt(1)
                # GEMM2 + gating scale -> ob [128, H] bf16, then scatter
                w2e = w2s[j]
                for rb in range(nblks[j]):
                    ob = obp.tile([128, 1, H], BF16, tag="ob")
                    for hc in range(H // 512):
                        p2 = ps.tile([128, 512], F32, tag="p2", bufs=3)
                        for f in range(FC):
                            nc.tensor.matmul(
                                p2[:],
                                hT[:, f, 128 * rb: 128 * (rb + 1)],
                                w2e[:, f, 512 * hc: 512 * (hc + 1)],
                                start=(f == 0), stop=(f == FC - 1),
                            )
                        nc.vector.tensor_scalar_mul(
                            ob[:, 0, 512 * hc: 512 * (hc + 1)], p2[:],
                            gat_t[:, boffs[j] + rb: boffs[j] + rb + 1],
                        )
                    nc.gpsimd.dma_scatter_add(
                        t_ymoe[:], ob[:],
                        idx_t[:, coffs[j] + 8 * rb: coffs[j] + 8 * (rb + 1)],
                        128, 128, H,
                    )

            # ---------------- shared expert (FS shard) ----------------
            sgt = shp.tile([128, HC, FSS], BF16, tag="sgt")
            sut = shp.tile([128, HC, FSS], BF16, tag="sut")
            sdt = shp.tile([128, FSS // 128, H], BF16, tag="sdt")
            for h2 in range(0, HC, 8):       # <=1.6us chunks
                nc.scalar.dma_start(sgt[:, h2: h2 + 8, :], t_sg[:, h2: h2 + 8, :])
                nc.scalar.dma_start(sut[:, h2: h2 + 8, :], t_su[:, h2: h2 + 8, :])
            for fs2 in range(FSS // 128):
                nc.scalar.dma_start(sdt[:, fs2: fs2 + 1, :], t_sd[:, fs2: fs2 + 1, :])
            for tb in range(N // 512):
                if tb + 2 < 4:
                    emit_xtt(tb + 2)
                xTt = xtts[tb]
                ttT = hp.tile([128, FSS // 128, 512], BF16, tag="ttT")
                for fs in range(FSS // 128):
                    pg = ps.tile([128, 512], F32, tag="p1")
                    pu = ps.tile([128, 512], F32, tag="p3")
                    for h in range(HC):
                        nc.tensor.matmul(
                            pg[:], sgt[:, h, 128 * fs: 128 * (fs + 1)],
                            xTt[:, h, :], start=(h == 0), stop=(h == HC - 1),
                        )
                    for h in range(HC):
                        nc.tensor.matmul(
                            pu[:], sut[:, h, 128 * fs: 128 * (fs + 1)],
                            xTt[:, h, :], start=(h == 0), stop=(h == HC - 1),
                        )
                    sig = hp.tile([128, 512], F32, tag="sigsh")
                    nc.scalar.activation(
                        sig[:], pg[:], mybir.ActivationFunctionType.Sigmoid
                    )
                    nc.vector.tensor_tensor(
                        sig[:], sig[:], pg[:], op=mybir.AluOpType.mult
                    )
                    nc.vector.tensor_tensor(
                        ttT[:, fs, :], sig[:], pu[:], op=mybir.AluOpType.mult
                    )
                for ts in range(4):
                    osh = obp.tile([128, 1, H], BF16, tag="ob")
                    for hc in range(H // 512):
                        p2 = ps.tile([128, 512], F32, tag="p2", bufs=3)
                        for fs in range(FSS // 128):
                            nc.tensor.matmul(
                                p2[:],
                                ttT[:, fs, 128 * ts: 128 * (ts + 1)],
                                sdt[:, fs, 512 * hc: 512 * (hc + 1)],
                                start=(fs == 0), stop=(fs == FSS // 128 - 1),
                            )
                        # split the PSUM drain across DVE and Act so the
                        # copy rate beats the GEMM produce rate
                        nc.vector.tensor_copy(
                            osh[:, 0, 512 * hc: 512 * hc + 256], p2[:, :256]
                        )
                        nc.scalar.activation(
                            osh[:, 0, 512 * hc + 256: 512 * (hc + 1)],
                            p2[:, 256:],
                            mybir.ActivationFunctionType.Copy,
                        )
                        # store each H-half as soon as its copies land
                        if hc % 2 == 1:
                            nc.sync.dma_start(
                                t_ysh[512 * tb + 128 * ts:
                                      512 * tb + 128 * (ts + 1),
                                      1024 * (hc // 2): 1024 * (hc // 2 + 1)],
                                osh[:, 0, 1024 * (hc // 2): 1024 * (hc // 2 + 1)],
                            )
    nc.compile()
    _CACHE[sched] = nc
    return nc


# ---------------------------------------------------------------- host glue
def _prep_inputs(hidden_states, gate_w, gate_bias, w1, w2, w3,
                 shared_gate, shared_up, shared_down):
    x = np.asarray(hidden_states, np.float32).reshape(N, H)

    def tile_lhsT(w):
        # [E, H, F] -> [E, FC, 128(h-part), HC*128]: tile (e,f)[p, hc*128+fo]
        # = w[e, 128*hc + p, 128*f + fo]
        we = w.reshape(-1, H // 128, 128, F // 128, 128)
        return np.ascontiguousarray(we.transpose(0, 3, 2, 1, 4)).reshape(
            -1, F // 128, 128, H)

    def tile_rhs(w, kc):
        # [E?, KC*128, M] -> [..., 128(part), KC, M]: (p, kc, m) = w[128*kc+p, m]
        wr = w.reshape(-1, kc, 128, w.shape[-1])
        return np.ascontiguousarray(wr.transpose(0, 2, 1, 3)).reshape(
            -1, 128, kc * w.shape[-1])

    topk_idx, topk_w = _gate_host_jax(x, np.asarray(gate_w, np.float32),
                                      np.asarray(gate_bias, np.float32))

    # capacity dispatch identical to reference: pos = per-expert running slot
    flat_e = topk_idx.reshape(-1)
    pos = np.zeros(N * K, np.int64)
    cnt = np.zeros(E, np.int64)
    for i, e in enumerate(flat_e):
        pos[i] = cnt[e]
        cnt[e] += 1
    keep = pos < CAP
    kept_counts = np.minimum(cnt, CAP)

    # Band schedule: sort experts by routed load (ascending, so the first
    # slot's gather is the smallest -> shortest startup), deal 8 per band
    # (one per core).  Static per-slot column cap = band max (padded to 4).
    order = np.argsort(kept_counts, kind="stable")
    scaps, nblks = [], []
    for j in range(EPC):
        cap = max(1, int(kept_counts[order[8 * j: 8 * (j + 1)]].max()))
        scaps.append(cap)
        nblks.append((cap + 127) // 128)
    sched = (tuple(scaps), tuple(nblks))
    NBT = sum(nblks)
    SLOTS_TOT = 128 * NBT

    x_nat = np.zeros((N + 1, H), BF)
    x_nat[:N] = x.astype(BF)
    xb = x.astype(BF)
    xT = tile_rhs(xb.T, H // 128)[0].reshape(128, H // 128, N)
    w1b = tile_lhsT(np.asarray(w1, np.float32).astype(BF))
    w3b = tile_lhsT(np.asarray(w3, np.float32).astype(BF))
    w2b = tile_rhs(np.asarray(w2, np.float32).astype(BF), F // 128).reshape(
        E, 128, F // 128, H)
    sgb = np.asarray(shared_gate, np.float32).astype(BF)
    sub = np.asarray(shared_up, np.float32).astype(BF)
    sdb = np.asarray(shared_down, np.float32).astype(BF)
    tw_flat = topk_w.reshape(-1)

    in_maps = []
    for c in range(NCORES):
        idx = np.full(SLOTS_TOT, N, np.int16)  # pads -> dump row N
        gat = np.zeros(SLOTS_TOT, np.float32)  # pads -> weight 0
        experts = []
        off = 0
        for j in range(EPC):
            eg = int(order[8 * j + c])
            experts.append(eg)
            sel = np.nonzero((flat_e == eg) & keep)[0]
            idx[off: off + len(sel)] = sel // K
            gat[off: off + len(sel)] = tw_flat[sel]
            off += 128 * nblks[j]
        lo = c * FSS
        in_maps.append({
            "x_nat": x_nat,
            "xT": xT,
            "idx_w": _wrap_idx(idx),
            "gat_col": gat.reshape(NBT, 128).T.copy(),
            "w1": np.ascontiguousarray(w1b[experts]),
            "w3": np.ascontiguousarray(w3b[experts]),
            "w2": np.ascontiguousarray(w2b[experts]),
            "sg": tile_rhs(sgb[:, lo:lo + FSS], H // 128)[0].reshape(128, H // 128, FSS),
            "su": tile_rhs(sub[:, lo:lo + FSS], H // 128)[0].reshape(128, H // 128, FSS),
            "sd": tile_rhs(sdb[lo:lo + FSS, :], FSS // 128)[0].reshape(128, FSS // 128, H),
        })
    return in_maps, sched


def kernel(hidden_states, gate_w, gate_bias, w1, w2, w3,
           shared_gate, shared_up, shared_down, _trace=False):
    in_maps, sched = _prep_inputs(hidden_states, gate_w, gate_bias, w1, w2, w3,
                                  shared_gate, shared_up, shared_down)
    nc = _build(sched)
    res = run_bass_kernel_spmd(nc, in_maps, list(range(NCORES)), trace=_trace)
    y = np.zeros((N, H), np.float64)
    for r in res.results:
        y += r["y_moe"][:N].astype(np.float64)
        y += r["y_sh"].astype(np.float64)
    out = y.astype(np.float32).reshape(B, S, H)
    if _trace:
        kernel._last = res
    return out


# revision 27
# speedup vs baseline: 1.0114x; 1.0013x over previous
"""KimiSparseMoeBlock kernel for 8 Trainium2 NeuronCores.

Sharding (expert-parallel, per spec hint):
  - 32 experts sharded 4-per-core (w1/w2/w3 leading dim), with the
    expert->core assignment chosen at runtime from the actual routed
    load: experts are sorted by token count and dealt into 4 "bands"
    (8 experts each, one per core).  Band j's static column capacity is
    the max count within the band, so the compiled SPMD program only
    computes (close to) the real routed tokens instead of the full
    2x-mean capacity buffer the reference uses.
  - Shared SwiGLU tensor-parallel along FS (2048/8 = 256 per core).
  - Token dispatch/combine done ON DEVICE via dma_gather(transpose) /
    dma_scatter_add with per-core index lists; gate/top-k routing
    metadata is computed host-side during input sharding (~0.1% FLOPs).
  - Each core emits two partial outputs (scatter-accumulated MoE rows
    and the FS-shard of the shared expert); unshard = sum of partials.

Device pipeline per core (per expert slot j with column cap S_j):
  gather ceil(S_j/128)*128 token rows (bf16, transposed to [H, W]) ->
  SwiGLU GEMMs on exactly S_j columns (bf16 PE, fp32 PSUM) ->
  gating-scaled rows -> dma_scatter_add into y_moe; plus FS-sharded
  shared SwiGLU from host-pretransposed xT.
"""
import os
import sys

sys.path.insert(0, "/opt/trn_rl_repo")

import numpy as np
import ml_dtypes

import concourse.bass as bass
import concourse.bacc as bacc
import concourse.tile as tile
import concourse.mybir as mybir
from concourse.bass_utils import run_bass_kernel_spmd

F32 = mybir.dt.float32
BF16 = mybir.dt.bfloat16
I16 = mybir.dt.int16
BF = ml_dtypes.bfloat16

E, K, G, TG = 32, 4, 4, 2
H, F, FS = 2048, 1024, 2048
SCALE = 2.0
B, S = 2, 1024
N = B * S
CAP = 2 * (N * K // E)  # 512 (reference capacity; tokens beyond it drop)
NCORES = 8
EPC = E // NCORES       # expert slots per core = 4
FSS = FS // NCORES      # shared intermediate per core = 256
HC = H // 128           # 16 contraction chunks
FC = F // 128           # 8 F chunks


# ---------------------------------------------------------------- routing
def _gate_host(x, gate_w, gate_bias):
    """Replicate reference _gate in numpy (f32, jax-compatible ops)."""
    x = x.astype(np.float32)
    logits = x @ gate_w.T.astype(np.float32)
    scores = 1.0 / (1.0 + np.exp(-logits))
    sc = scores + gate_bias[None, :]
    n = x.shape[0]
    grp = sc.reshape(n, G, E // G)
    top2 = np.sort(grp, axis=-1)[:, :, -2:]
    group_scores = top2.sum(-1)
    gidx = np.argsort(-group_scores, axis=-1, kind="stable")[:, :TG]
    gmask = np.zeros((n, G), np.float32)
    np.put_along_axis(gmask, gidx, 1.0, axis=1)
    smask = np.repeat(gmask, E // G, axis=1)
    tmp = np.where(smask > 0, sc, 0.0)
    topk_idx = np.argsort(-tmp, axis=-1, kind="stable")[:, :K].astype(np.int32)
    topk_w = np.take_along_axis(scores, topk_idx, axis=1)
    topk_w = topk_w / (topk_w.sum(-1, keepdims=True) + 1e-20)
    return topk_idx, (topk_w * SCALE).astype(np.float32)


def _gate_host_jax(x, gate_w, gate_bias):
    """Bit-exact replication of the reference gate on jax-cpu."""
    try:
        import jax

        import jax.numpy as jnp

        def gate(x, gate_w, gate_bias):
            logits = x @ gate_w.T
            scores = jax.nn.sigmoid(logits)
            sc = scores + gate_bias
            n = x.shape[0]
            grp = sc.reshape(n, G, E // G)
            group_scores = jax.lax.top_k(grp, 2)[0].sum(-1)
            _, gidx = jax.lax.top_k(group_scores, TG)
            gmask = (
                jnp.zeros((n, G), sc.dtype)
                .at[jnp.arange(n)[:, None], gidx]
                .set(1.0)
            )
            smask = jnp.repeat(gmask, E // G, axis=1)
            tmp = jnp.where(smask > 0, sc, 0.0)
            _, topk_idx = jax.lax.top_k(tmp, K)
            topk_w = jnp.take_along_axis(scores, topk_idx, axis=1)
            topk_w = topk_w / (topk_w.sum(-1, keepdims=True) + 1e-20)
            return topk_idx, topk_w * SCALE

        with jax.default_device(jax.devices("cpu")[0]):
            ti, tw = jax.jit(gate, backend="cpu")(x, gate_w, gate_bias)
        return np.asarray(ti, np.int32), np.asarray(tw, np.float32)
    except Exception:
        return _gate_host(x, gate_w, gate_bias)


def _wrap_idx(idx):
    """[n] -> [128, n//16]: slot s at partition s%16 (replicated x8), col s//16."""
    n = idx.shape[0]
    w = idx.reshape(n // 16, 16).T
    return np.tile(w, (8, 1)).copy()


# ---------------------------------------------------------------- bass build
_CACHE = {}


def _build(sched):
    """sched = (scaps, nblks): static per-slot column caps / 128-blocks."""
    if sched in _CACHE:
        return _CACHE[sched]
    scaps, nblks = sched
    NBT = sum(nblks)           # total 128-row token blocks per core
    SLOTS_TOT = 128 * NBT

    nc = bacc.Bacc("TRN2", target_bir_lowering=False, debug=False,
                   num_devices=NCORES)
    t_x = nc.dram_tensor("x_nat", [N + 1, H], BF16, kind="ExternalInput")
    t_xT = nc.dram_tensor("xT", [128, H // 128, N], BF16, kind="ExternalInput")
    t_idx = nc.dram_tensor("idx_w", [128, SLOTS_TOT // 16], I16,
                           kind="ExternalInput")
    t_gat = nc.dram_tensor("gat_col", [128, NBT], F32, kind="ExternalInput")
    t_w1 = nc.dram_tensor("w1", [EPC, F // 128, 128, H], BF16, kind="ExternalInput")
    t_w3 = nc.dram_tensor("w3", [EPC, F // 128, 128, H], BF16, kind="ExternalInput")
    t_w2 = nc.dram_tensor("w2", [EPC, 128, F // 128, H], BF16, kind="ExternalInput")
    t_sg = nc.dram_tensor("sg", [128, H // 128, FSS], BF16, kind="ExternalInput")
    t_su = nc.dram_tensor("su", [128, H // 128, FSS], BF16, kind="ExternalInput")
    t_sd = nc.dram_tensor("sd", [128, FSS // 128, H], BF16, kind="ExternalInput")
    t_ymoe = nc.dram_tensor("y_moe", [N + 1, H], BF16, kind="ExternalOutput")
    t_ysh = nc.dram_tensor("y_sh", [N, H], BF16, kind="ExternalOutput")

    coffs = np.cumsum([0] + [8 * nb for nb in nblks]).tolist()   # idx col offs
    boffs = np.cumsum([0] + list(nblks)).tolist()                # block offs

    with tile.TileContext(nc) as tc:
        with (
            tc.tile_pool(name="idxp", bufs=1) as idxp,
            tc.tile_pool(name="gath", bufs=2) as gath,
            tc.tile_pool(name="wp", bufs=3) as wp,
            tc.tile_pool(name="w2p", bufs=2) as w2p,
            tc.tile_pool(name="hp", bufs=2) as hp,
            tc.tile_pool(name="obp", bufs=2) as obp,
            tc.tile_pool(name="shp", bufs=1) as shp,
            tc.tile_pool(name="ps", bufs=2, space="PSUM") as ps,
        ):
            # idx on the pool queue (its consumer): the first gather starts
            # as soon as it lands instead of queueing behind w2 chunks.
            idx_t = idxp.tile([128, SLOTS_TOT // 16], I16)
            nc.gpsimd.dma_start(idx_t[:], t_idx[:])
            gat_t = idxp.tile([128, NBT], F32)   # load deferred (needed ~30us in)

            # warm the sigmoid act-table during an Act-queue idle window so
            # the 1.3us table load doesn't land mid-pipeline at first use
            with tc.tile_wait_until(0.005):
                warm = idxp.tile([128, 1], F32)
                nc.vector.memset(warm[:], 0.0)
                warm2 = idxp.tile([128, 1], F32)
                nc.scalar.activation(warm2[:], warm[:],
                                     mybir.ActivationFunctionType.Sigmoid)

            bufs = [None] * EPC   # gathered token tiles (list per slot)
            w2s = [None] * EPC    # w2 weight tiles
            xtts = [None] * 4     # shared-path xT tiles

            def emit_gather(j, split=1):
                # split>1: issue the gather as `split` pieces so the first
                # GEMM can start before the whole slot is resident.
                W = 128 * nblks[j]
                tiles = []
                step = W // split
                assert step % 128 == 0
                for s in range(split):
                    t = gath.tile([128, HC, step], BF16, tag="bufT",
                                  name=f"bufT{j}_{s}")
                    c0 = coffs[j] + s * step // 16
                    nc.gpsimd.dma_gather(
                        t[:], t_x[:], idx_t[:, c0: c0 + step // 16],
                        step, step, H, transpose=True,
                    )
                    tiles.append(t)
                bufs[j] = tiles

            def emit_w2(j, split=1):
                # chunked per-f so urgent ops never queue behind a 4MB DMA;
                # split=2 halves the chunks again (startup-critical window)
                w2s[j] = w2p.tile([128, FC, H], BF16, tag="w2e",
                                  name=f"w2e{j}")
                hstep = H // split
                for f in range(FC):
                    for s in range(split):
                        nc.gpsimd.dma_start(
                            w2s[j][:, f, hstep * s: hstep * (s + 1)],
                            t_w2[j, :, f, hstep * s: hstep * (s + 1)],
                        )

            def emit_xtt(tb):
                # shares the bufT tag: expert gathers are dead by the time
                # each xT tile loads, so the same two buffers cycle on
                xtts[tb] = gath.tile([128, HC, 512], BF16, tag="bufT",
                                     name=f"xTt{tb}")
                for s in range(4):
                    nc.sync.dma_start(
                        xtts[tb][:, :, 128 * s: 128 * (s + 1)],
                        t_xT[:, :, 512 * tb + 128 * s: 512 * tb + 128 * (s + 1)],
                    )

            # Prefetch: slot 0/1 gathers + slot 0/1 w2 up-front (pool queue);
            # slot j+2's gather/w2 are emitted at slot j+1's head so they sit
            # behind slot j's scatters but well ahead of their consumers.
            with tc.high_priority():
                emit_gather(0, split=2 if nblks[0] >= 2 else 1)
                emit_gather(1)
            # hold the w2 prefetch off the pool queue until the first
            # gather (the PE-critical op) has had its chance to run
            with tc.tile_wait_until(0.0025):
                emit_w2(0, split=2)
            emit_w2(1)

            # ---------------- expert path ----------------
            for j in range(EPC):
                Sj = scaps[j]
                W = 128 * nblks[j]
                if j >= 1 and j + 1 < EPC:
                    emit_gather(j + 1)
                    emit_w2(j + 1)
                # clip gather pieces to the Sj real columns
                pieces = []
                off = 0
                for t in bufs[j]:
                    w = t.shape[2]
                    lo = off
                    hi = min(off + w, Sj)
                    if hi > lo:
                        pieces.append((t, lo, hi - lo))
                    off += w
                # GEMM1 + GEMM3 + SwiGLU -> hT [128, FC, W] bf16
                hT = hp.tile([128, FC, W], BF16, tag="hT", name=f"hT{j}")
                if Sj < W:
                    nc.vector.memset(hT[:, :, Sj:], 0.0)
                for f in range(FC):
                    w1f = wp.tile([128, HC, 128], BF16, tag="w1f")
                    w3f = wp.tile([128, HC, 128], BF16, tag="w3f")
                    # first w1 of the run goes via SP — the Act queue opens
                    # with a 1.3us act-table load
                    w1eng = nc.sync if (j == 0 and f == 0) else nc.scalar
                    w1eng.dma_start(
                        w1f[:], t_w1[j, f].rearrange("p (hc fo) -> p hc fo", hc=HC)
                    )
                    nc.sync.dma_start(
                        w3f[:], t_w3[j, f].rearrange("p (hc fo) -> p hc fo", hc=HC)
                    )
                    p1 = ps.tile([128, Sj], F32, tag="p1")
                    p3 = ps.tile([128, Sj], F32, tag="p3")
                    for (t, lo, n) in pieces:
                        for h in range(HC):
                            nc.tensor.matmul(p1[:, lo: lo + n], w1f[:, h, :],
                                             t[:, h, :n],
                                             start=(h == 0), stop=(h == HC - 1))
                    for (t, lo, n) in pieces:
                        for h in range(HC):
                            nc.tensor.matmul(p3[:, lo: lo + n], w3f[:, h, :],
                                             t[:, h, :n],
                                             start=(h == 0), stop=(h == HC - 1))
                    sig = hp.tile([128, Sj], F32, tag="sig")
                    nc.scalar.activation(
                        sig[:], p1[:], mybir.ActivationFunctionType.Sigmoid
                    )
                    nc.vector.tensor_tensor(
                        sig[:], sig[:], p1[:], op=mybir.AluOpType.mult
                    )
                    nc.vector.tensor_tensor(
                        hT[:, f, :Sj], sig[:], p3[:], op=mybir.AluOpType.mult
                    )
                if j == 0:
                    nc.sync.dma_start(gat_t[:], t_gat[:])
                # prefetch shared-path xT during the last slot's GEMM2
                if j == EPC - 1:
                    emit_xtt(0)
                    emit_xtt(1)
                # GEMM2 + gating scale -> ob [128, H] bf16, then scatter
                w2e = w2s[j]
                for rb in range(nblks[j]):
                    ob = obp.tile([128, 1, H], BF16, tag="ob")
                    for hc in range(H // 512):
                        p2 = ps.tile([128, 512], F32, tag="p2", bufs=3)
                        for f in range(FC):
                            nc.tensor.matmul(
                                p2[:],
                                hT[:, f, 128 * rb: 128 * (rb + 1)],
                                w2e[:, f, 512 * hc: 512 * (hc + 1)],
                                start=(f == 0), stop=(f == FC - 1),
                            )
                        nc.vector.tensor_scalar_mul(
                            ob[:, 0, 512 * hc: 512 * (hc + 1)], p2[:],
                            gat_t[:, boffs[j] + rb: boffs[j] + rb + 1],
                        )
                    nc.gpsimd.dma_scatter_add(
                        t_ymoe[:], ob[:],
                        idx_t[:, coffs[j] + 8 * rb: coffs[j] + 8 * (rb + 1)],
                        128, 128, H,
                    )

            # ---------------- shared expert (FS shard) ----------------
            sgt = shp.tile([128, HC, FSS], BF16, tag="sgt")
            sut = shp.tile([128, HC, FSS], BF16, tag="sut")
            sdt = shp.tile([128, FSS // 128, H], BF16, tag="sdt")
            for h2 in range(0, HC, 8):       # <=1.6us chunks
                nc.scalar.dma_start(sgt[:, h2: h2 + 8, :], t_sg[:, h2: h2 + 8, :])
                nc.scalar.dma_start(sut[:, h2: h2 + 8, :], t_su[:, h2: h2 + 8, :])
            for fs2 in range(FSS // 128):
                nc.scalar.dma_start(sdt[:, fs2: fs2 + 1, :], t_sd[:, fs2: fs2 + 1, :])
            for tb in range(N // 512):
                if tb + 2 < 4:
                    emit_xtt(tb + 2)
                xTt = xtts[tb]
                ttT = hp.tile([128, FSS // 128, 512], BF16, tag="ttT")
                for fs in range(FSS // 128):
                    pg = ps.tile([128, 512], F32, tag="p1")
                    pu = ps.tile([128, 512], F32, tag="p3")
                    for h in range(HC):
                        nc.tensor.matmul(
                            pg[:], sgt[:, h, 128 * fs: 128 * (fs + 1)],
                            xTt[:, h, :], start=(h == 0), stop=(h == HC - 1),
                        )
                    for h in range(HC):
                        nc.tensor.matmul(
                            pu[:], sut[:, h, 128 * fs: 128 * (fs + 1)],
                            xTt[:, h, :], start=(h == 0), stop=(h == HC - 1),
                        )
                    sig = hp.tile([128, 512], F32, tag="sigsh")
                    nc.scalar.activation(
                        sig[:], pg[:], mybir.ActivationFunctionType.Sigmoid
                    )
                    nc.vector.tensor_tensor(
                        sig[:], sig[:], pg[:], op=mybir.AluOpType.mult
                    )
                    nc.vector.tensor_tensor(
                        ttT[:, fs, :], sig[:], pu[:], op=mybir.AluOpType.mult
                    )
                for ts in range(4):
                    osh = obp.tile([128, 1, H], BF16, tag="ob")
                    for hc in range(H // 512):
                        p2 = ps.tile([128, 512], F32, tag="p2", bufs=3)
                        for fs in range(FSS // 128):
                            nc.tensor.matmul(
                                p2[:],
                                ttT[:, fs, 128 * ts: 128 * (ts + 1)],
                                sdt[:, fs, 512 * hc: 512 * (hc + 1)],
                                start=(fs == 0), stop=(fs == FSS // 128 - 1),
                            )
                        # split the PSUM drain across DVE and Act so the
                        # copy rate beats the GEMM produce rate
                        nc.vector.tensor_copy(
                            osh[:, 0, 512 * hc: 512 * hc + 256], p2[:, :256]
                        )
                        nc.scalar.activation(
                            osh[:, 0, 512 * hc + 256: 512 * (hc + 1)],
                            p2[:, 256:],
                            mybir.ActivationFunctionType.Copy,
                        )
                        # store each H-half as soon as its copies land; in
                        # the final block store per-chunk so the tail drains
                        # as early as possible
                        if tb == N // 512 - 1:
                            nc.sync.dma_start(
                                t_ysh[512 * tb + 128 * ts:
                                      512 * tb + 128 * (ts + 1),
                                      512 * hc: 512 * (hc + 1)],
                                osh[:, 0, 512 * hc: 512 * (hc + 1)],
                            )
                        elif hc % 2 == 1:
                            nc.sync.dma_start(
                                t_ysh[512 * tb + 128 * ts:
                                      512 * tb + 128 * (ts + 1),
                                      1024 * (hc // 2): 1024 * (hc // 2 + 1)],
                                osh[:, 0, 1024 * (hc // 2): 1024 * (hc // 2 + 1)],
                            )
    nc.compile()
    _CACHE[sched] = nc
    return nc


# ---------------------------------------------------------------- host glue
def _prep_inputs(hidden_states, gate_w, gate_bias, w1, w2, w3,
                 shared_gate, shared_up, shared_down):
    x = np.asarray(hidden_states, np.float32).reshape(N, H)

    def tile_lhsT(w):
        # [E, H, F] -> [E, FC, 128(h-part), HC*128]: tile (e,f)[p, hc*128+fo]
        # = w[e, 128*hc + p, 128*f + fo]
        we = w.reshape(-1, H // 128, 128, F // 128, 128)
        return np.ascontiguousarray(we.transpose(0, 3, 2, 1, 4)).reshape(
            -1, F // 128, 128, H)

    def tile_rhs(w, kc):
        # [E?, KC*128, M] -> [..., 128(part), KC, M]: (p, kc, m) = w[128*kc+p, m]
        wr = w.reshape(-1, kc, 128, w.shape[-1])
        return np.ascontiguousarray(wr.transpose(0, 2, 1, 3)).reshape(
            -1, 128, kc * w.shape[-1])

    topk_idx, topk_w = _gate_host_jax(x, np.asarray(gate_w, np.float32),
                                      np.asarray(gate_bias, np.float32))

    # capacity dispatch identical to reference: pos = per-expert running slot
    flat_e = topk_idx.reshape(-1)
    pos = np.zeros(N * K, np.int64)
    cnt = np.zeros(E, np.int64)
    for i, e in enumerate(flat_e):
        pos[i] = cnt[e]
        cnt[e] += 1
    keep = pos < CAP
    kept_counts = np.minimum(cnt, CAP)

    # Band schedule: sort experts by routed load (ascending, so the first
    # slot's gather is the smallest -> shortest startup), deal 8 per band
    # (one per core).  Static per-slot column cap = band max (padded to 4).
    order = np.argsort(kept_counts, kind="stable")
    scaps, nblks = [], []
    for j in range(EPC):
        cap = max(1, int(kept_counts[order[8 * j: 8 * (j + 1)]].max()))
        scaps.append(cap)
        nblks.append((cap + 127) // 128)
    sched = (tuple(scaps), tuple(nblks))
    NBT = sum(nblks)
    SLOTS_TOT = 128 * NBT

    x_nat = np.zeros((N + 1, H), BF)
    x_nat[:N] = x.astype(BF)
    xb = x.astype(BF)
    xT = tile_rhs(xb.T, H // 128)[0].reshape(128, H // 128, N)
    w1b = tile_lhsT(np.asarray(w1, np.float32).astype(BF))
    w3b = tile_lhsT(np.asarray(w3, np.float32).astype(BF))
    w2b = tile_rhs(np.asarray(w2, np.float32).astype(BF), F // 128).reshape(
        E, 128, F // 128, H)
    sgb = np.asarray(shared_gate, np.float32).astype(BF)
    sub = np.asarray(shared_up, np.float32).astype(BF)
    sdb = np.asarray(shared_down, np.float32).astype(BF)
    tw_flat = topk_w.reshape(-1)

    in_maps = []
    for c in range(NCORES):
        idx = np.full(SLOTS_TOT, N, np.int16)  # pads -> dump row N
        gat = np.zeros(SLOTS_TOT, np.float32)  # pads -> weight 0
        experts = []
        off = 0
        for j in range(EPC):
            eg = int(order[8 * j + c])
            experts.append(eg)
            sel = np.nonzero((flat_e == eg) & keep)[0]
            idx[off: off + len(sel)] = sel // K
            gat[off: off + len(sel)] = tw_flat[sel]
            off += 128 * nblks[j]
        lo = c * FSS
        in_maps.append({
            "x_nat": x_nat,
            "xT": xT,
            "idx_w": _wrap_idx(idx),
            "gat_col": gat.reshape(NBT, 128).T.copy(),
            "w1": np.ascontiguousarray(w1b[experts]),
            "w3": np.ascontiguousarray(w3b[experts]),
            "w2": np.ascontiguousarray(w2b[experts]),
            "sg": tile_rhs(sgb[:, lo:lo + FSS], H // 128)[0].reshape(128, H // 128, FSS),
            "su": tile_rhs(sub[:, lo:lo + FSS], H // 128)[0].reshape(128, H // 128, FSS),
            "sd": tile_rhs(sdb[lo:lo + FSS, :], FSS // 128)[0].reshape(128, FSS // 128, H),
        })
    return in_maps, sched


def kernel(hidden_states, gate_w, gate_bias, w1, w2, w3,
           shared_gate, shared_up, shared_down, _trace=False):
    in_maps, sched = _prep_inputs(hidden_states, gate_w, gate_bias, w1, w2, w3,
                                  shared_gate, shared_up, shared_down)
    nc = _build(sched)
    res = run_bass_kernel_spmd(nc, in_maps, list(range(NCORES)), trace=_trace)
    y = np.zeros((N, H), np.float64)
    for r in res.results:
        y += r["y_moe"][:N].astype(np.float64)
        y += r["y_sh"].astype(np.float64)
    out = y.astype(np.float32).reshape(B, S, H)
    if _trace:
        kernel._last = res
    return out
